# revision 3
# baseline (speedup 1.0000x reference)
"""Trainium2 Bass kernel for nn_AttentionPool (gnn_message_passing).

Strategy (v2: class-group windowed pooling)
-------------------------------------------
Math restructure (exactly equivalent to the reference up to fp rounding):
  score[n,h] = context_h[n,:] @ V[:,h] + c[h]        (fold W_lin/W_att/b_lin/b_att;
               V[k,h] = sum_o W_lin[h*128+o, k] * W_att[o],  c[h] = b_lin[h*128:].W_att + b_att)
  p = exp(leaky_relu(score, 0.2))                     (skip segment-max: scores are
                                                       O(1), exp cannot overflow; softmax
                                                       is shift-invariant so result is
                                                       identical up to rounding)
  denom[cls,h]  = sum_{n: y=cls} p[n,h]
  pooled[cls,h,:] = sum_{n: y=cls} p[n,h]*context_h[n,:] / denom[cls,h]

Sharding: BY CLASS. Host argsorts context_y; core k owns classes
[125k, 125k+125) -> no cross-core reduction.

Key idea vs v1: within a core, classes are split into 4 GROUPS of <=32
consecutive classes, and the (sorted) nodes of each group are padded to a
whole number of 128-node tiles. Every tile therefore touches only one
32-class window, so the scatter-add matmul can run "flipped":

  per tile j (128 nodes), group g:
    o_w4[n, h*32+w] = (yg[n]==w) * p[n,h]         4 narrow DVE ops (FD=32)
    acc_g[c,    hw] += hn_j[:, 0:128].T @ o_w4    matmul, feat chunk 0
    acc_g[c+.., hw] += hn_j[:,128:256].T @ o_w4   matmul, feat chunk 1
    acc_g[0(d), hw] += ones.T        @ o_w4       denom row
  (acc_g lives in one PSUM bank: cols 0:128 chunk0, 128:256 chunk1,
   256:384 denom row; flushed to SBUF by ACT when the group ends.)

The moving operand is now 128 cols/tile instead of 4*257, the one-hot DVE
ops are 32 wide instead of 128, and scores keep the v1 structure. Host
divides pooled by denom and reassembles.
"""

import sys

sys.path.insert(0, "/opt/trn_rl_repo")

import numpy as np
import ml_dtypes

BF = ml_dtypes.bfloat16

N = 100000
INC = 256
NHEAD = 4
OUTC = 128
NCLS = 1000
NCORES = 8
CPC = NCLS // NCORES  # 125 classes per core
GCLS = 32  # classes per group window
NGRP = 4  # groups per core (32+32+32+29)
W4 = NHEAD * GCLS  # 128 = pooling matmul moving width
BANKW = 512  # one PSUM bank (f32 cols); 0:128 c0, 128:256 c1, 256:384 denom

_PROG_CACHE = {}
LAST_RESULT = None
LAST_PROFILE = None


def build_program(tg, s_tiles=16):
    """Build + compile the SPMD Bass program. tg = tiles per group (len 4)."""
    from concourse import bacc, mybir, tile

    f32 = mybir.dt.float32
    bf16 = mybir.dt.bfloat16
    AF = mybir.ActivationFunctionType
    OP = mybir.AluOpType

    t_tiles = int(sum(tg))
    cap = t_tiles * 128
    # tile index -> (group, first-in-group, last-in-group)
    grp_of = []
    for g, tcnt in enumerate(tg):
        for i in range(tcnt):
            grp_of.append((g, i == 0, i == tcnt - 1))

    nc = bacc.Bacc(
        "TRN2", target_bir_lowering=False, debug=False, num_devices=NCORES
    )

    hn = nc.dram_tensor("hn", [128, t_tiles, INC], bf16, kind="ExternalInput").ap()
    htr = nc.dram_tensor("htr", [128, 2, cap], bf16, kind="ExternalInput").ap()
    yg = nc.dram_tensor("yg", [128, t_tiles], f32, kind="ExternalInput").ap()
    w_lin = nc.dram_tensor("w_lin", [NHEAD * OUTC, INC], f32, kind="ExternalInput").ap()
    b_lin_r = nc.dram_tensor("b_lin_r", [OUTC, NHEAD], f32, kind="ExternalInput").ap()
    w_att = nc.dram_tensor("w_att", [OUTC, 1], f32, kind="ExternalInput").ap()
    b_att = nc.dram_tensor("b_att", [1, 1], f32, kind="ExternalInput").ap()
    ciota = nc.dram_tensor("ciota", [128, GCLS], bf16, kind="ExternalInput").ap()
    out = nc.dram_tensor(
        "outp", [128, NGRP * 384], f32, kind="ExternalOutput"
    ).ap()

    nsup = (t_tiles + s_tiles - 1) // s_tiles

    def sup_range(s_):
        ts0 = s_ * s_tiles
        return ts0, min(s_tiles, t_tiles - ts0)

    with tile.TileContext(nc) as tc:
        with (
            tc.tile_pool(name="const", bufs=1) as cpool,
            tc.tile_pool(name="stream", bufs=3) as sb,
            tc.tile_pool(name="work", bufs=2) as sg,
            tc.tile_pool(name="ow", bufs=12) as owp,
            tc.tile_pool(name="ps", bufs=2, space="PSUM") as ps,
            tc.tile_pool(name="acc", bufs=2, space="PSUM") as accp,
        ):
            # ---- constants -------------------------------------------------
            ciota_sb = cpool.tile([128, GCLS], bf16)
            nc.sync.dma_start(out=ciota_sb[:], in_=ciota)
            watt_sb = cpool.tile([128, 1], f32)
            nc.sync.dma_start(out=watt_sb[:], in_=w_att)
            blin_sb = cpool.tile([128, NHEAD], f32)
            nc.sync.dma_start(out=blin_sb[:], in_=b_lin_r)
            batt_sb = cpool.tile([1, 1], f32)
            nc.sync.dma_start(out=batt_sb[:1], in_=b_att)
            ones_row = cpool.tile([1, 128], bf16)
            nc.vector.memset(ones_row[:1], 1.0)
            ones_col = cpool.tile([128, 1], bf16)
            nc.vector.memset(ones_col[:], 1.0)

            # ---- fold W_lin/W_att into V [256,4] (two 128-chunks), c [1,4] --
            v_bf = []
            for ch in range(2):
                v_ps = ps.tile([128, NHEAD], f32, tag="sps")
                for h in range(NHEAD):
                    wl = sg.tile([128, 128], f32, tag="wl")
                    nc.sync.dma_start(
                        out=wl[:],
                        in_=w_lin[h * 128 : (h + 1) * 128, ch * 128 : (ch + 1) * 128],
                    )
                    nc.tensor.matmul(
                        v_ps[:, h : h + 1], lhsT=wl[:], rhs=watt_sb[:],
                        start=True, stop=True,
                    )
                vb = cpool.tile([128, NHEAD], bf16, tag=f"vbf{ch}")
                nc.vector.tensor_copy(out=vb[:], in_=v_ps[:])
                v_bf.append(vb)

            c_ps = ps.tile([1, NHEAD], f32, tag="sps")
            nc.tensor.matmul(c_ps[:1], lhsT=watt_sb[:], rhs=blin_sb[:],
                             start=True, stop=True)
            c_bf = cpool.tile([1, NHEAD], bf16)
            nc.scalar.activation(c_bf[:1], c_ps[:1], AF.Identity,
                                 bias=batt_sb[:1, :1])
            c_rep = cpool.tile([1, s_tiles * NHEAD], bf16)
            for r in range(s_tiles):
                nc.vector.tensor_copy(
                    out=c_rep[:1, r * NHEAD : (r + 1) * NHEAD], in_=c_bf[:1, :]
                )

            # ---- main pipelined loop --------------------------------------
            out_sb = cpool.tile([128, NGRP * 384], f32)

            stream_tiles = {}
            p_tiles = {}
            acc_state = {"t": None}

            def load_and_scores(s_):
                ts0, nst = sup_range(s_)
                c0 = ts0 * 128
                c1 = c0 + nst * 128
                # h^T chunks on the ACT HWDGE ring; node-major on the SP ring
                ht = sb.tile([128, 2, s_tiles * 128], bf16, tag="ht")
                nc.scalar.dma_start(out=ht[:, :, : nst * 128], in_=htr[:, :, c0:c1])
                hns = sb.tile([128, s_tiles, INC], bf16, tag="hn")
                nc.sync.dma_start(out=hns[:, :nst, :], in_=hn[:, ts0 : ts0 + nst, :])
                yt = sb.tile([128, s_tiles], f32, tag="yt")
                nc.sync.dma_start(out=yt[:, :nst], in_=yg[:, ts0 : ts0 + nst])
                stream_tiles[s_] = (ht, hns, yt, nst)

                # scores for the whole supertile into one PSUM bank
                s_ps = ps.tile([128, s_tiles * NHEAD], f32, tag="sps")
                nw = nst * NHEAD
                nc.tensor.matmul(
                    s_ps[:, :nw], lhsT=ones_row[:1], rhs=c_rep[:1, :nw],
                    start=True, stop=False, skip_group_check=True,
                )
                for j in range(nst):
                    sl = slice(j * NHEAD, (j + 1) * NHEAD)
                    nc.tensor.matmul(
                        s_ps[:, sl], lhsT=ht[:, 0, j * 128 : (j + 1) * 128],
                        rhs=v_bf[0][:], start=False, stop=False,
                        skip_group_check=True,
                    )
                    nc.tensor.matmul(
                        s_ps[:, sl], lhsT=ht[:, 1, j * 128 : (j + 1) * 128],
                        rhs=v_bf[1][:], start=False, stop=True,
                        skip_group_check=True,
                    )
                # p = exp(leaky_relu(s)); leaky(x) = max(x, 0.2x)
                t02 = sg.tile([128, s_tiles * NHEAD], f32, tag="t02")
                nc.vector.tensor_scalar_mul(t02[:, :nw], s_ps[:, :nw], 0.2)
                slr = sg.tile([128, s_tiles * NHEAD], f32, tag="slr")
                nc.vector.tensor_tensor(
                    out=slr[:, :nw], in0=s_ps[:, :nw], in1=t02[:, :nw], op=OP.max
                )
                p_sb = sg.tile([128, s_tiles * NHEAD], f32, tag="p")
                nc.scalar.activation(p_sb[:, :nw], slr[:, :nw], AF.Exp)
                p_tiles[s_] = p_sb

            def pools(s_):
                ht, hns, yt, nst = stream_tiles.pop(s_)
                p_sb = p_tiles.pop(s_)
                ts0, _ = sup_range(s_)
                for j in range(nst):
                    t_ = ts0 + j
                    g, first, last = grp_of[t_]
                    if first:
                        acc_state["t"] = accp.tile(
                            [128, BANKW], f32, tag="gacc", name="gacc"
                        )
                    acc = acc_state["t"]
                    o_w4 = owp.tile([128, W4], bf16, tag="ow")
                    for h in range(NHEAD):
                        nc.vector.tensor_scalar(
                            out=o_w4[:, h * GCLS : (h + 1) * GCLS],
                            in0=ciota_sb[:],
                            scalar1=yt[:, j : j + 1],
                            scalar2=p_sb[:, j * NHEAD + h : j * NHEAD + h + 1],
                            op0=OP.is_equal, op1=OP.mult,
                        )
                    # feat chunk 0 / chunk 1 / denom row, one PSUM bank
                    nc.tensor.matmul(
                        acc[:, 0:128], lhsT=hns[:, j, 0:128], rhs=o_w4[:],
                        start=first, stop=last, skip_group_check=True,
                    )
                    nc.tensor.matmul(
                        acc[:, 128:256], lhsT=hns[:, j, 128:256], rhs=o_w4[:],
                        start=False, stop=last, skip_group_check=True,
                    )
                    nc.tensor.matmul(
                        acc[0:1, 256:384], lhsT=ones_col[:], rhs=o_w4[:],
                        start=False, stop=last, skip_group_check=True,
                    )
                    if last:
                        nc.scalar.activation(
                            out_sb[:, g * 384 : (g + 1) * 384],
                            acc[:, 0:384], AF.Copy,
                        )

            load_and_scores(0)
            for s_ in range(nsup):
                if s_ + 1 < nsup:
                    load_and_scores(s_ + 1)
                pools(s_)

            nc.sync.dma_start(out=out, in_=out_sb[:])

    nc.compile()
    return nc


def _prep_inputs(context_h, W_lin, b_lin, W_att, b_att, context_y):
    """Host-side shard: argsort by class, split into 8 class ranges of 125,
    sub-split into 4 groups of <=32 classes, pad each group's nodes to whole
    128-node tiles (group tile counts uniform across cores)."""
    h = np.asarray(context_h, dtype=np.float32)
    hb = h.astype(BF)
    y = np.asarray(context_y).astype(np.int64)
    order = np.argsort(y, kind="stable")
    ys = y[order]

    # group boundaries: 32 per-core groups of <=GCLS classes
    gb = []
    for k in range(NCORES):
        for g in range(NGRP):
            gb.append(k * CPC + min(g * GCLS, CPC))
    gb.append(NCLS)
    gbounds = np.searchsorted(ys, np.asarray(gb))
    cnts = (gbounds[1:] - gbounds[:-1]).reshape(NCORES, NGRP)
    tg = tuple(int(-(-int(cnts[:, g].max()) // 128)) for g in range(NGRP))
    t_tiles = int(sum(tg))

    W_lin = np.ascontiguousarray(np.asarray(W_lin, dtype=np.float32))
    b_lin_r = np.ascontiguousarray(
        np.asarray(b_lin, dtype=np.float32).reshape(NHEAD, OUTC).T
    )
    w_att = np.asarray(W_att, dtype=np.float32).reshape(OUTC, 1)
    b_att = np.asarray(b_att, dtype=np.float32).reshape(1, 1)
    ciota = np.ascontiguousarray(
        np.broadcast_to(np.arange(GCLS, dtype=np.float32), (128, GCLS))
    ).astype(BF)

    in_maps = []
    for k in range(NCORES):
        hp = np.zeros((t_tiles * 128, INC), dtype=BF)
        ygp = np.full((t_tiles * 128,), float(GCLS + 95), dtype=np.float32)
        row = 0
        for g in range(NGRP):
            gi = k * NGRP + g
            lo, hi = gbounds[gi], gbounds[gi + 1]
            cnt = hi - lo
            idx = order[lo:hi]
            hp[row : row + cnt] = hb[idx]
            ygp[row : row + cnt] = (ys[lo:hi] - (k * CPC + g * GCLS)).astype(
                np.float32
            )
            row += tg[g] * 128
        hn = np.ascontiguousarray(
            hp.reshape(t_tiles, 128, INC).transpose(1, 0, 2)
        )
        htr = np.ascontiguousarray(
            hp.reshape(t_tiles * 128, 2, 128).transpose(2, 1, 0)
        )
        ygt = np.ascontiguousarray(ygp.reshape(t_tiles, 128).T)
        in_maps.append(
            {
                "hn": hn,
                "htr": htr,
                "yg": ygt,
                "w_lin": W_lin,
                "b_lin_r": b_lin_r,
                "w_att": w_att,
                "b_att": b_att,
                "ciota": ciota,
            }
        )
    return in_maps, tg, cnts, gbounds, ys


def kernel(context_h, W_lin, b_lin, W_att, b_att, context_y, num_classes):
    global LAST_RESULT, LAST_PROFILE
    import os

    assert int(num_classes) == NCLS

    from concourse.bass_utils import run_bass_kernel_spmd

    in_maps, tg, cnts, gbounds, ys = _prep_inputs(
        context_h, W_lin, b_lin, W_att, b_att, context_y
    )
    if tg not in _PROG_CACHE:
        _PROG_CACHE[tg] = build_program(tg)
    nc = _PROG_CACHE[tg]
    core_ids = list(range(NCORES))
    res = run_bass_kernel_spmd(nc, in_maps, core_ids)
    LAST_RESULT = res

    if os.environ.get("KERNEL_PROFILE") == "1":
        LAST_PROFILE = run_bass_kernel_spmd(nc, in_maps, core_ids, trace=True)

    outp = np.empty((NCLS, NHEAD * INC), dtype=np.float32)
    for k in range(NCORES):
        o = np.asarray(res.results[k]["outp"])  # [128, 4*384]
        for g in range(NGRP):
            blk = o[:, g * 384 : (g + 1) * 384]
            ncls_g = min(GCLS, CPC - g * GCLS)
            # blk[:, hw] chunk0: [c0..127, (h,w)]; chunk1 at cols 128:256;
            # denom row 0 at cols 256:384
            c0 = blk[:, 0:128].reshape(128, NHEAD, GCLS)
            c1 = blk[:, 128:256].reshape(128, NHEAD, GCLS)
            den = blk[0, 256:384].reshape(NHEAD, GCLS)
            den = np.where(den != 0.0, den, 1.0)
            pooled = np.concatenate([c0, c1], axis=0)  # [256, h, w]
            pooled = pooled / den[None, :, :]
            # -> [w, h, c]
            pw = pooled.transpose(2, 1, 0).reshape(GCLS, NHEAD * INC)
            base = k * CPC + g * GCLS
            outp[base : base + ncls_g] = pw[:ncls_g]
    return outp


# revision 6
# speedup vs baseline: 1.3549x; 1.3549x over previous
"""Trainium2 Bass kernel for nn_AttentionPool (gnn_message_passing).

Strategy (v2: class-group windowed pooling)
-------------------------------------------
Math restructure (exactly equivalent to the reference up to fp rounding):
  score[n,h] = context_h[n,:] @ V[:,h] + c[h]        (fold W_lin/W_att/b_lin/b_att;
               V[k,h] = sum_o W_lin[h*128+o, k] * W_att[o],  c[h] = b_lin[h*128:].W_att + b_att)
  p = exp(leaky_relu(score, 0.2))                     (skip segment-max: scores are
                                                       O(1), exp cannot overflow; softmax
                                                       is shift-invariant so result is
                                                       identical up to rounding)
  denom[cls,h]  = sum_{n: y=cls} p[n,h]
  pooled[cls,h,:] = sum_{n: y=cls} p[n,h]*context_h[n,:] / denom[cls,h]

Sharding: BY CLASS. Host argsorts context_y; core k owns classes
[125k, 125k+125) -> no cross-core reduction.

Key idea vs v1: within a core, classes are split into 4 GROUPS of <=32
consecutive classes, and the (sorted) nodes of each group are padded to a
whole number of 128-node tiles. Every tile therefore touches only one
32-class window, so the scatter-add matmul can run "flipped":

  per tile j (128 nodes), group g:
    o_w4[n, h*32+w] = (yg[n]==w) * p[n,h]         4 narrow DVE ops (FD=32)
    acc_g[c,    hw] += hn_j[:, 0:128].T @ o_w4    matmul, feat chunk 0
    acc_g[c+.., hw] += hn_j[:,128:256].T @ o_w4   matmul, feat chunk 1
    acc_g[0(d), hw] += ones.T        @ o_w4       denom row
  (acc_g lives in one PSUM bank: cols 0:128 chunk0, 128:256 chunk1,
   256:384 denom row; flushed to SBUF by ACT when the group ends.)

The moving operand is now 128 cols/tile instead of 4*257, the one-hot DVE
ops are 32 wide instead of 128, and scores keep the v1 structure. Host
divides pooled by denom and reassembles.
"""

import sys

sys.path.insert(0, "/opt/trn_rl_repo")

import numpy as np
import ml_dtypes

BF = ml_dtypes.bfloat16

N = 100000
INC = 256
NHEAD = 4
OUTC = 128
NCLS = 1000
NCORES = 8
CPC = NCLS // NCORES  # 125 classes per core
GCLS = 32  # classes per group window
NGRP = 4  # groups per core (32+32+32+29)
W4 = NHEAD * GCLS  # 128 = pooling matmul moving width
BANKW = 512  # one PSUM bank (f32 cols); 0:128 c0, 128:256 c1, 256:384 denom

_PROG_CACHE = {}
LAST_RESULT = None
LAST_PROFILE = None


def build_program(tg, s_tiles=16):
    """Build + compile the SPMD Bass program. tg = tiles per group (len 4)."""
    from concourse import bacc, mybir, tile

    f32 = mybir.dt.float32
    bf16 = mybir.dt.bfloat16
    AF = mybir.ActivationFunctionType
    OP = mybir.AluOpType

    t_tiles = int(sum(tg))
    cap = t_tiles * 128
    # tile index -> (group, first-in-group, last-in-group)
    grp_of = []
    for g, tcnt in enumerate(tg):
        for i in range(tcnt):
            grp_of.append((g, i == 0, i == tcnt - 1))

    nc = bacc.Bacc(
        "TRN2", target_bir_lowering=False, debug=False, num_devices=NCORES
    )

    hn = nc.dram_tensor("hn", [128, t_tiles, INC], bf16, kind="ExternalInput").ap()
    htr = nc.dram_tensor("htr", [128, 2, cap], bf16, kind="ExternalInput").ap()
    yg = nc.dram_tensor("yg", [128, t_tiles], f32, kind="ExternalInput").ap()
    w_lin = nc.dram_tensor("w_lin", [NHEAD * OUTC, INC], f32, kind="ExternalInput").ap()
    b_lin_r = nc.dram_tensor("b_lin_r", [OUTC, NHEAD], f32, kind="ExternalInput").ap()
    w_att = nc.dram_tensor("w_att", [OUTC, 1], f32, kind="ExternalInput").ap()
    b_att = nc.dram_tensor("b_att", [1, 1], f32, kind="ExternalInput").ap()
    ciota = nc.dram_tensor("ciota", [128, GCLS], bf16, kind="ExternalInput").ap()
    out = nc.dram_tensor(
        "outp", [128, NGRP * 384], f32, kind="ExternalOutput"
    ).ap()

    nsup = (t_tiles + s_tiles - 1) // s_tiles

    def sup_range(s_):
        ts0 = s_ * s_tiles
        return ts0, min(s_tiles, t_tiles - ts0)

    with tile.TileContext(nc) as tc:
        with (
            tc.tile_pool(name="const", bufs=1) as cpool,
            tc.tile_pool(name="stream", bufs=3) as sb,
            tc.tile_pool(name="work", bufs=2) as sg,
            tc.tile_pool(name="ow", bufs=12) as owp,
            tc.tile_pool(name="ps", bufs=2, space="PSUM") as ps,
            tc.tile_pool(name="acc", bufs=2, space="PSUM") as accp,
        ):
            # ---- constants -------------------------------------------------
            ciota_sb = cpool.tile([128, GCLS], bf16)
            nc.sync.dma_start(out=ciota_sb[:], in_=ciota)
            watt_sb = cpool.tile([128, 1], f32)
            nc.sync.dma_start(out=watt_sb[:], in_=w_att)
            blin_sb = cpool.tile([128, NHEAD], f32)
            nc.sync.dma_start(out=blin_sb[:], in_=b_lin_r)
            batt_sb = cpool.tile([1, 1], f32)
            nc.sync.dma_start(out=batt_sb[:1], in_=b_att)
            ones_row = cpool.tile([1, 128], bf16)
            nc.vector.memset(ones_row[:1], 1.0)
            ones_col = cpool.tile([128, 1], bf16)
            nc.vector.memset(ones_col[:], 1.0)

            # ---- fold W_lin/W_att into V [256,4] (two 128-chunks), c [1,4] --
            v_bf = []
            for ch in range(2):
                v_ps = ps.tile([128, NHEAD], f32, tag="sps")
                for h in range(NHEAD):
                    wl = sg.tile([128, 128], f32, tag="wl")
                    nc.sync.dma_start(
                        out=wl[:],
                        in_=w_lin[h * 128 : (h + 1) * 128, ch * 128 : (ch + 1) * 128],
                    )
                    nc.tensor.matmul(
                        v_ps[:, h : h + 1], lhsT=wl[:], rhs=watt_sb[:],
                        start=True, stop=True,
                    )
                vb = cpool.tile([128, NHEAD], bf16, tag=f"vbf{ch}")
                nc.vector.tensor_copy(out=vb[:], in_=v_ps[:])
                v_bf.append(vb)

            c_ps = ps.tile([1, NHEAD], f32, tag="sps")
            nc.tensor.matmul(c_ps[:1], lhsT=watt_sb[:], rhs=blin_sb[:],
                             start=True, stop=True)
            c_bf = cpool.tile([1, NHEAD], bf16)
            nc.scalar.activation(c_bf[:1], c_ps[:1], AF.Identity,
                                 bias=batt_sb[:1, :1])
            c_rep = cpool.tile([1, s_tiles * NHEAD], bf16)
            for r in range(s_tiles):
                nc.vector.tensor_copy(
                    out=c_rep[:1, r * NHEAD : (r + 1) * NHEAD], in_=c_bf[:1, :]
                )

            # ---- main pipelined loop --------------------------------------
            out_sb = cpool.tile([128, NGRP * 384], f32)

            stream_tiles = {}
            p_tiles = {}
            acc_state = {"t": None}

            def load_and_scores(s_):
                ts0, nst = sup_range(s_)
                c0 = ts0 * 128
                c1 = c0 + nst * 128
                # h^T chunks on the ACT HWDGE ring; node-major on the SP ring
                ht = sb.tile([128, 2, s_tiles * 128], bf16, tag="ht")
                nc.scalar.dma_start(out=ht[:, :, : nst * 128], in_=htr[:, :, c0:c1])
                hns = sb.tile([128, s_tiles, INC], bf16, tag="hn")
                nc.sync.dma_start(out=hns[:, :nst, :], in_=hn[:, ts0 : ts0 + nst, :])
                yt = sb.tile([128, s_tiles], f32, tag="yt")
                nc.sync.dma_start(out=yt[:, :nst], in_=yg[:, ts0 : ts0 + nst])
                stream_tiles[s_] = (ht, hns, yt, nst)

                # scores for the whole supertile into one PSUM bank
                s_ps = ps.tile([128, s_tiles * NHEAD], f32, tag="sps")
                nw = nst * NHEAD
                nc.tensor.matmul(
                    s_ps[:, :nw], lhsT=ones_row[:1], rhs=c_rep[:1, :nw],
                    start=True, stop=False, skip_group_check=True,
                )
                for j in range(nst):
                    sl = slice(j * NHEAD, (j + 1) * NHEAD)
                    nc.tensor.matmul(
                        s_ps[:, sl], lhsT=ht[:, 0, j * 128 : (j + 1) * 128],
                        rhs=v_bf[0][:], start=False, stop=False,
                        skip_group_check=True,
                    )
                    nc.tensor.matmul(
                        s_ps[:, sl], lhsT=ht[:, 1, j * 128 : (j + 1) * 128],
                        rhs=v_bf[1][:], start=False, stop=True,
                        skip_group_check=True,
                    )
                # p = exp(leaky_relu(s, 0.2)) -- leaky on ACT, then the exp
                # broadcasts p across the 32-class window (stride-0 read) so
                # the weighted one-hot needs just two wide DVE ops/supertile.
                t02 = sg.tile([128, s_tiles * NHEAD], f32, tag="t02")
                nc.vector.tensor_scalar_mul(t02[:, :nw], s_ps[:, :nw], 0.2)
                slr = sg.tile([128, s_tiles * NHEAD], f32, tag="slr")
                nc.vector.tensor_tensor(
                    out=slr[:, :nw], in0=s_ps[:, :nw], in1=t02[:, :nw], op=OP.max
                )
                mask = sg.tile([128, s_tiles, GCLS], bf16, tag="mask")
                nc.vector.tensor_tensor(
                    out=mask[:, :nst, :],
                    in0=yt[:, :nst].to_broadcast([128, nst, GCLS]),
                    in1=ciota_sb[:]
                    .to_broadcast([128, GCLS, nst])
                    .rearrange("p w j -> p j w"),
                    op=OP.is_equal,
                )
                prep = sg.tile([128, s_tiles, NHEAD, GCLS], bf16, tag="prep")
                nc.scalar.activation(
                    prep[:, :nst],
                    slr[:, :nw]
                    .rearrange("p (j h) -> p j h", h=NHEAD)
                    .to_broadcast([128, nst, NHEAD, GCLS]),
                    AF.Exp,
                )
                ow = sb.tile([128, s_tiles, NHEAD, GCLS], bf16, tag="ow")
                nc.vector.tensor_tensor(
                    out=ow[:, :nst],
                    in0=mask[:, :nst, :]
                    .to_broadcast([128, nst, GCLS, NHEAD])
                    .rearrange("p j w h -> p j h w"),
                    in1=prep[:, :nst],
                    op=OP.mult,
                )
                p_tiles[s_] = ow

            def pools(s_):
                ht, hns, yt, nst = stream_tiles.pop(s_)
                ow = p_tiles.pop(s_)
                ts0, _ = sup_range(s_)
                for j in range(nst):
                    t_ = ts0 + j
                    g, first, last = grp_of[t_]
                    if first:
                        acc_state["t"] = accp.tile(
                            [128, BANKW], f32, tag="gacc", name="gacc"
                        )
                    acc = acc_state["t"]
                    rhs = ow[:, j]
                    # feat chunk 0 / chunk 1 / denom row, one PSUM bank
                    nc.tensor.matmul(
                        acc[:, 0:128], lhsT=hns[:, j, 0:128], rhs=rhs,
                        start=first, stop=last, skip_group_check=True,
                    )
                    nc.tensor.matmul(
                        acc[:, 128:256], lhsT=hns[:, j, 128:256], rhs=rhs,
                        start=False, stop=last, skip_group_check=True,
                    )
                    nc.tensor.matmul(
                        acc[0:1, 256:384], lhsT=ones_col[:], rhs=rhs,
                        start=False, stop=last, skip_group_check=True,
                    )
                    if last:
                        nc.scalar.activation(
                            out_sb[:, g * 384 : (g + 1) * 384],
                            acc[:, 0:384], AF.Copy,
                        )

            load_and_scores(0)
            for s_ in range(nsup):
                if s_ + 1 < nsup:
                    load_and_scores(s_ + 1)
                pools(s_)

            nc.sync.dma_start(out=out, in_=out_sb[:])

    nc.compile()
    return nc


def _prep_inputs(context_h, W_lin, b_lin, W_att, b_att, context_y):
    """Host-side shard: argsort by class, split into 8 class ranges of 125,
    sub-split into 4 groups of <=32 classes, pad each group's nodes to whole
    128-node tiles (group tile counts uniform across cores)."""
    h = np.asarray(context_h, dtype=np.float32)
    hb = h.astype(BF)
    y = np.asarray(context_y).astype(np.int64)
    order = np.argsort(y, kind="stable")
    ys = y[order]

    # group boundaries: 32 per-core groups of <=GCLS classes
    gb = []
    for k in range(NCORES):
        for g in range(NGRP):
            gb.append(k * CPC + min(g * GCLS, CPC))
    gb.append(NCLS)
    gbounds = np.searchsorted(ys, np.asarray(gb))
    cnts = (gbounds[1:] - gbounds[:-1]).reshape(NCORES, NGRP)
    tg = tuple(int(-(-int(cnts[:, g].max()) // 128)) for g in range(NGRP))
    t_tiles = int(sum(tg))

    W_lin = np.ascontiguousarray(np.asarray(W_lin, dtype=np.float32))
    b_lin_r = np.ascontiguousarray(
        np.asarray(b_lin, dtype=np.float32).reshape(NHEAD, OUTC).T
    )
    w_att = np.asarray(W_att, dtype=np.float32).reshape(OUTC, 1)
    b_att = np.asarray(b_att, dtype=np.float32).reshape(1, 1)
    ciota = np.ascontiguousarray(
        np.broadcast_to(np.arange(GCLS, dtype=np.float32), (128, GCLS))
    ).astype(BF)

    in_maps = []
    for k in range(NCORES):
        hp = np.zeros((t_tiles * 128, INC), dtype=BF)
        ygp = np.full((t_tiles * 128,), float(GCLS + 95), dtype=np.float32)
        row = 0
        for g in range(NGRP):
            gi = k * NGRP + g
            lo, hi = gbounds[gi], gbounds[gi + 1]
            cnt = hi - lo
            idx = order[lo:hi]
            hp[row : row + cnt] = hb[idx]
            ygp[row : row + cnt] = (ys[lo:hi] - (k * CPC + g * GCLS)).astype(
                np.float32
            )
            row += tg[g] * 128
        hn = np.ascontiguousarray(
            hp.reshape(t_tiles, 128, INC).transpose(1, 0, 2)
        )
        htr = np.ascontiguousarray(
            hp.reshape(t_tiles * 128, 2, 128).transpose(2, 1, 0)
        )
        ygt = np.ascontiguousarray(ygp.reshape(t_tiles, 128).T)
        in_maps.append(
            {
                "hn": hn,
                "htr": htr,
                "yg": ygt,
                "w_lin": W_lin,
                "b_lin_r": b_lin_r,
                "w_att": w_att,
                "b_att": b_att,
                "ciota": ciota,
            }
        )
    return in_maps, tg, cnts, gbounds, ys


def kernel(context_h, W_lin, b_lin, W_att, b_att, context_y, num_classes):
    global LAST_RESULT, LAST_PROFILE
    import os

    assert int(num_classes) == NCLS

    from concourse.bass_utils import run_bass_kernel_spmd

    in_maps, tg, cnts, gbounds, ys = _prep_inputs(
        context_h, W_lin, b_lin, W_att, b_att, context_y
    )
    if tg not in _PROG_CACHE:
        _PROG_CACHE[tg] = build_program(tg)
    nc = _PROG_CACHE[tg]
    core_ids = list(range(NCORES))
    res = run_bass_kernel_spmd(nc, in_maps, core_ids)
    LAST_RESULT = res

    if os.environ.get("KERNEL_PROFILE") == "1":
        LAST_PROFILE = run_bass_kernel_spmd(nc, in_maps, core_ids, trace=True)

    outp = np.empty((NCLS, NHEAD * INC), dtype=np.float32)
    for k in range(NCORES):
        o = np.asarray(res.results[k]["outp"])  # [128, 4*384]
        for g in range(NGRP):
            blk = o[:, g * 384 : (g + 1) * 384]
            ncls_g = min(GCLS, CPC - g * GCLS)
            # blk[:, hw] chunk0: [c0..127, (h,w)]; chunk1 at cols 128:256;
            # denom row 0 at cols 256:384
            c0 = blk[:, 0:128].reshape(128, NHEAD, GCLS)
            c1 = blk[:, 128:256].reshape(128, NHEAD, GCLS)
            den = blk[0, 256:384].reshape(NHEAD, GCLS)
            den = np.where(den != 0.0, den, 1.0)
            pooled = np.concatenate([c0, c1], axis=0)  # [256, h, w]
            pooled = pooled / den[None, :, :]
            # -> [w, h, c]
            pw = pooled.transpose(2, 1, 0).reshape(GCLS, NHEAD * INC)
            base = k * CPC + g * GCLS
            outp[base : base + ncls_g] = pw[:ncls_g]
    return outp


# revision 8
# speedup vs baseline: 1.5774x; 1.1642x over previous
"""Trainium2 Bass kernel for nn_AttentionPool (gnn_message_passing).

Strategy (v4: class-group windowed pooling, supertile-batched one-hot)
----------------------------------------------------------------------
Math restructure (exactly equivalent to the reference up to fp rounding):
  score[n,h] = context_h[n,:] @ V[:,h] + c[h]        (V, c host-folded from
               W_lin/W_att/b_lin/b_att: V[k,h] = sum_o W_lin[h*128+o,k]W_att[o],
               c[h] = b_lin[h*128:(h+1)*128].W_att + b_att)
  p = exp(leaky_relu(score, 0.2))                     (skip segment-max: scores
               are O(1) so exp cannot overflow; softmax is shift-invariant)
  denom[cls,h]  = sum_{n: y=cls} p[n,h]
  pooled[cls,h,:] = sum_{n: y=cls} p[n,h]*context_h[n,:] / denom[cls,h]

Sharding: BY CLASS. Host argsorts context_y; core k owns classes
[125k, 125k+125) -> no cross-core reduction.

Within a core, classes split into 4 GROUPS of <=32 consecutive classes; the
sorted nodes of each group are padded to whole 128-node tiles, so every tile
touches one 32-class window and the scatter-add matmul runs "flipped":

  per tile j (128 nodes) in group g:
    ow[n, h*32+w] = (yg[n]==w) * p[n,h]
    acc_g[c,    hw] += hn_j[:, 0:128].T @ ow      (feat chunk 0)
    acc_g[c+.., hw] += hn_j[:,128:256].T @ ow     (feat chunk 1)
    acc_g[0(d), hw] += ones.T         @ ow        (denom row)
  acc_g = one PSUM bank (cols 0:128 chunk0, 128:256 chunk1, 256:384 denom);
  ACT flushes it to SBUF at group end, then it's DMA'd out immediately.

ow is built per 16-tile supertile in 2 wide DVE ops (+1 ACT op):
  mask[n,j,w] = (yg[n,j] == iota[w])        one TT is_equal w/ broadcast APs
  prep[n,j,h,w] = Exp(slr[n,j,h])           ACT exp, stride-0 bcast over w
  ow = mask (bcast over h) * prep           one TT mult
which sidesteps the ~190ns fixed cost of narrow per-tile DVE ops.

DMA: host pre-lays-out everything contiguous; hn carries yg as column 256;
h^T goes on the ACT HWDGE ring, hn on the SP ring; constants are one DMA;
each group's result is DMA'd out as soon as it's flushed.
"""

import sys

sys.path.insert(0, "/opt/trn_rl_repo")

import numpy as np
import ml_dtypes

BF = ml_dtypes.bfloat16

N = 100000
INC = 256
NHEAD = 4
OUTC = 128
NCLS = 1000
NCORES = 8
CPC = NCLS // NCORES  # 125 classes per core
GCLS = 32  # classes per group window
NGRP = 4  # groups per core (32+32+32+29)
NCOL = INC + 2  # hn row: 256 features, yg, zero pad
BANKW = 512  # one PSUM bank; 0:128 c0, 128:256 c1, 256:384 denom

_PROG_CACHE = {}
LAST_RESULT = None
LAST_PROFILE = None


def build_program(tg, s_tiles=16):
    """Build + compile the SPMD Bass program. tg = tiles per group (len 4)."""
    from concourse import bacc, mybir, tile

    f32 = mybir.dt.float32
    bf16 = mybir.dt.bfloat16
    AF = mybir.ActivationFunctionType
    OP = mybir.AluOpType

    t_tiles = int(sum(tg))
    cap = t_tiles * 128
    grp_of = []
    for g, tcnt in enumerate(tg):
        for i in range(tcnt):
            grp_of.append((g, i == 0, i == tcnt - 1))

    nc = bacc.Bacc(
        "TRN2", target_bir_lowering=False, debug=False, num_devices=NCORES
    )

    hn = nc.dram_tensor("hn", [128, t_tiles, NCOL], bf16, kind="ExternalInput").ap()
    htr = nc.dram_tensor("htr", [128, 2, cap], bf16, kind="ExternalInput").ap()
    # consts: cols 0:32 iota, 32:36 V chunk0, 36:40 V chunk1, row0 40:44 c
    cst = nc.dram_tensor("cst", [128, 44], bf16, kind="ExternalInput").ap()
    out = nc.dram_tensor(
        "outp", [128, NGRP * 384], f32, kind="ExternalOutput"
    ).ap()

    nsup = (t_tiles + s_tiles - 1) // s_tiles

    def sup_range(s_):
        ts0 = s_ * s_tiles
        return ts0, min(s_tiles, t_tiles - ts0)

    with tile.TileContext(nc) as tc:
        with (
            tc.tile_pool(name="const", bufs=1) as cpool,
            tc.tile_pool(name="stream", bufs=3) as sb,
            tc.tile_pool(name="work", bufs=2) as sg,
            tc.tile_pool(name="ps", bufs=3, space="PSUM") as ps,
            tc.tile_pool(name="acc", bufs=2, space="PSUM") as accp,
        ):
            # ---- constants (one DMA) --------------------------------------
            cst_sb = cpool.tile([128, 44], bf16)
            nc.sync.dma_start(out=cst_sb[:], in_=cst)
            ciota_sb = cst_sb[:, 0:GCLS]
            v_bf = [cst_sb[:, 32:36], cst_sb[:, 36:40]]
            ones_row = cpool.tile([1, 128], bf16)
            nc.vector.memset(ones_row[:1], 1.0)
            ones_col = cpool.tile([128, 1], bf16)
            nc.vector.memset(ones_col[:], 1.0)
            c_rep = cpool.tile([1, s_tiles * NHEAD], bf16)
            nc.vector.tensor_copy(
                out=c_rep[:1].rearrange("p (j h) -> p j h", h=NHEAD),
                in_=cst_sb[0:1, 40:44]
                .to_broadcast([1, NHEAD, s_tiles])
                .rearrange("p h j -> p j h"),
            )

            out_sb = cpool.tile([128, NGRP * 384], f32)

            stream_tiles = {}
            ow_tiles = {}
            acc_state = {"t": None}

            def load_and_scores(s_):
                ts0, nst = sup_range(s_)
                c0 = ts0 * 128
                c1 = c0 + nst * 128
                # h^T chunks on the ACT HWDGE ring; node-major on the SP ring
                ht = sb.tile([128, 2, s_tiles * 128], bf16, tag="ht")
                nc.scalar.dma_start(out=ht[:, :, : nst * 128], in_=htr[:, :, c0:c1])
                hns = sb.tile([128, s_tiles, NCOL], bf16, tag="hn")
                nc.sync.dma_start(out=hns[:, :nst, :], in_=hn[:, ts0 : ts0 + nst, :])
                stream_tiles[s_] = (ht, hns, nst)

                # scores for the whole supertile into one PSUM bank
                s_ps = ps.tile([128, s_tiles * NHEAD], f32, tag="sps")
                nw = nst * NHEAD
                nc.tensor.matmul(
                    s_ps[:, :nw], lhsT=ones_row[:1], rhs=c_rep[:1, :nw],
                    start=True, stop=False, skip_group_check=True,
                )
                for j in range(nst):
                    sl = slice(j * NHEAD, (j + 1) * NHEAD)
                    nc.tensor.matmul(
                        s_ps[:, sl], lhsT=ht[:, 0, j * 128 : (j + 1) * 128],
                        rhs=v_bf[0], start=False, stop=False,
                        skip_group_check=True,
                    )
                    nc.tensor.matmul(
                        s_ps[:, sl], lhsT=ht[:, 1, j * 128 : (j + 1) * 128],
                        rhs=v_bf[1], start=False, stop=True,
                        skip_group_check=True,
                    )
                # p = exp(leaky_relu(s, 0.2)); exp broadcasts p across the
                # 32-class window (stride-0 read) so the weighted one-hot is
                # two wide DVE ops per supertile.
                t02 = sg.tile([128, s_tiles * NHEAD], f32, tag="t02")
                nc.vector.tensor_scalar_mul(t02[:, :nw], s_ps[:, :nw], 0.2)
                slr = sg.tile([128, s_tiles * NHEAD], f32, tag="slr")
                nc.vector.tensor_tensor(
                    out=slr[:, :nw], in0=s_ps[:, :nw], in1=t02[:, :nw], op=OP.max
                )
                mask = sg.tile([128, s_tiles, GCLS], bf16, tag="mask")
                nc.vector.tensor_tensor(
                    out=mask[:, :nst, :],
                    in0=hns[:, :nst, INC].to_broadcast([128, nst, GCLS]),
                    in1=ciota_sb.to_broadcast([128, GCLS, nst]).rearrange(
                        "p w j -> p j w"
                    ),
                    op=OP.is_equal,
                )
                prep = sg.tile([128, s_tiles, NHEAD, GCLS], bf16, tag="prep")
                nc.scalar.activation(
                    prep[:, :nst],
                    slr[:, :nw]
                    .rearrange("p (j h) -> p j h", h=NHEAD)
                    .to_broadcast([128, nst, NHEAD, GCLS]),
                    AF.Exp,
                )
                ow = sb.tile([128, s_tiles, NHEAD, GCLS], bf16, tag="ow")
                nc.vector.tensor_tensor(
                    out=ow[:, :nst],
                    in0=mask[:, :nst, :]
                    .to_broadcast([128, nst, GCLS, NHEAD])
                    .rearrange("p j w h -> p j h w"),
                    in1=prep[:, :nst],
                    op=OP.mult,
                )
                ow_tiles[s_] = ow

            def pools(s_):
                ht, hns, nst = stream_tiles.pop(s_)
                ow = ow_tiles.pop(s_)
                ts0, _ = sup_range(s_)
                for j in range(nst):
                    t_ = ts0 + j
                    g, first, last = grp_of[t_]
                    if first:
                        acc_state["t"] = accp.tile(
                            [128, BANKW], f32, tag="gacc", name="gacc"
                        )
                    acc = acc_state["t"]
                    rhs = ow[:, j]
                    nc.tensor.matmul(
                        acc[:, 0:128], lhsT=hns[:, j, 0:128], rhs=rhs,
                        start=first, stop=last, skip_group_check=True,
                    )
                    nc.tensor.matmul(
                        acc[:, 128:256], lhsT=hns[:, j, 128:256], rhs=rhs,
                        start=False, stop=last, skip_group_check=True,
                    )
                    nc.tensor.matmul(
                        acc[0:1, 256:384], lhsT=ones_col[:], rhs=rhs,
                        start=False, stop=last, skip_group_check=True,
                    )
                    if last:
                        nc.scalar.activation(
                            out_sb[:, g * 384 : (g + 1) * 384],
                            acc[:, 0:384], AF.Copy,
                        )
                        nc.sync.dma_start(
                            out=out[:, g * 384 : (g + 1) * 384],
                            in_=out_sb[:, g * 384 : (g + 1) * 384],
                        )

            load_and_scores(0)
            if nsup > 1:
                load_and_scores(1)
            for s_ in range(nsup):
                if s_ + 2 < nsup:
                    load_and_scores(s_ + 2)
                pools(s_)

    nc.compile()
    return nc


def _prep_inputs(context_h, W_lin, b_lin, W_att, b_att, context_y):
    """Host-side shard: argsort by class, 8 class ranges of 125, each split
    into 4 groups of <=32 classes, nodes padded to whole 128-node tiles."""
    h = np.asarray(context_h, dtype=np.float32)
    hb = h.astype(BF)
    y = np.asarray(context_y).astype(np.int64)
    order = np.argsort(y, kind="stable")
    ys = y[order]

    gb = []
    for k in range(NCORES):
        for g in range(NGRP):
            gb.append(k * CPC + min(g * GCLS, CPC))
    gb.append(NCLS)
    gbounds = np.searchsorted(ys, np.asarray(gb))
    cnts = (gbounds[1:] - gbounds[:-1]).reshape(NCORES, NGRP)
    tg = tuple(int(-(-int(cnts[:, g].max()) // 128)) for g in range(NGRP))
    t_tiles = int(sum(tg))

    # fold V[k,h], c[h] from W_lin/W_att/b_lin/b_att (weights-only preproc)
    W_lin = np.asarray(W_lin, dtype=np.float32)
    W_att = np.asarray(W_att, dtype=np.float32)
    V = np.einsum("hok,o->kh", W_lin.reshape(NHEAD, OUTC, INC), W_att)  # [256,4]
    c = (
        np.asarray(b_lin, dtype=np.float32).reshape(NHEAD, OUTC) @ W_att
        + np.asarray(b_att, dtype=np.float32)[0]
    )  # [4]
    cst = np.zeros((128, 44), dtype=BF)
    cst[:, 0:GCLS] = np.arange(GCLS, dtype=np.float32)[None, :].astype(BF)
    cst[:, 32:36] = V[0:128].astype(BF)
    cst[:, 36:40] = V[128:256].astype(BF)
    cst[0, 40:44] = c.astype(BF)

    in_maps = []
    for k in range(NCORES):
        hp = np.zeros((t_tiles * 128, NCOL), dtype=BF)
        hp[:, INC] = BF(float(GCLS + 95))  # pad marker 127 -> mask==0
        row = 0
        for g in range(NGRP):
            gi = k * NGRP + g
            lo, hi = gbounds[gi], gbounds[gi + 1]
            cnt = hi - lo
            idx = order[lo:hi]
            hp[row : row + cnt, 0:INC] = hb[idx]
            hp[row : row + cnt, INC] = (
                ys[lo:hi] - (k * CPC + g * GCLS)
            ).astype(np.float32).astype(BF)
            row += tg[g] * 128
        hn = np.ascontiguousarray(
            hp.reshape(t_tiles, 128, NCOL).transpose(1, 0, 2)
        )
        htr = np.ascontiguousarray(
            hp[:, 0:INC].reshape(t_tiles * 128, 2, 128).transpose(2, 1, 0)
        )
        in_maps.append({"hn": hn, "htr": htr, "cst": cst})
    return in_maps, tg


def kernel(context_h, W_lin, b_lin, W_att, b_att, context_y, num_classes):
    global LAST_RESULT, LAST_PROFILE
    import os

    assert int(num_classes) == NCLS

    from concourse.bass_utils import run_bass_kernel_spmd

    in_maps, tg = _prep_inputs(
        context_h, W_lin, b_lin, W_att, b_att, context_y
    )
    if tg not in _PROG_CACHE:
        _PROG_CACHE[tg] = build_program(tg)
    nc = _PROG_CACHE[tg]
    core_ids = list(range(NCORES))
    res = run_bass_kernel_spmd(nc, in_maps, core_ids)
    LAST_RESULT = res

    if os.environ.get("KERNEL_PROFILE") == "1":
        LAST_PROFILE = run_bass_kernel_spmd(nc, in_maps, core_ids, trace=True)

    outp = np.empty((NCLS, NHEAD * INC), dtype=np.float32)
    for k in range(NCORES):
        o = np.asarray(res.results[k]["outp"])  # [128, 4*384]
        for g in range(NGRP):
            blk = o[:, g * 384 : (g + 1) * 384]
            ncls_g = min(GCLS, CPC - g * GCLS)
            c0 = blk[:, 0:128].reshape(128, NHEAD, GCLS)
            c1 = blk[:, 128:256].reshape(128, NHEAD, GCLS)
            den = blk[0, 256:384].reshape(NHEAD, GCLS)
            den = np.where(den != 0.0, den, 1.0)
            pooled = np.concatenate([c0, c1], axis=0)  # [256, h, w]
            pooled = pooled / den[None, :, :]
            pw = pooled.transpose(2, 1, 0).reshape(GCLS, NHEAD * INC)
            base = k * CPC + g * GCLS
            outp[base : base + ncls_g] = pw[:ncls_g]
    return outp


# revision 11
# speedup vs baseline: 1.6161x; 1.0246x over previous
"""Trainium2 Bass kernel for nn_AttentionPool (gnn_message_passing).

Strategy (v4: class-group windowed pooling, supertile-batched one-hot)
----------------------------------------------------------------------
Math restructure (exactly equivalent to the reference up to fp rounding):
  score[n,h] = context_h[n,:] @ V[:,h] + c[h]        (V, c host-folded from
               W_lin/W_att/b_lin/b_att: V[k,h] = sum_o W_lin[h*128+o,k]W_att[o],
               c[h] = b_lin[h*128:(h+1)*128].W_att + b_att)
  p = exp(leaky_relu(score, 0.2))                     (skip segment-max: scores
               are O(1) so exp cannot overflow; softmax is shift-invariant)
  denom[cls,h]  = sum_{n: y=cls} p[n,h]
  pooled[cls,h,:] = sum_{n: y=cls} p[n,h]*context_h[n,:] / denom[cls,h]

Sharding: BY CLASS. Host argsorts context_y; core k owns classes
[125k, 125k+125) -> no cross-core reduction.

Within a core, classes split into 4 GROUPS of <=32 consecutive classes; the
sorted nodes of each group are padded to whole 128-node tiles, so every tile
touches one 32-class window and the scatter-add matmul runs "flipped":

  per tile j (128 nodes) in group g:
    ow[n, h*32+w] = (yg[n]==w) * p[n,h]
    acc_g[c,    hw] += hn_j[:, 0:128].T @ ow      (feat chunk 0)
    acc_g[c+.., hw] += hn_j[:,128:256].T @ ow     (feat chunk 1)
    acc_g[0(d), hw] += ones.T         @ ow        (denom row)
  acc_g = one PSUM bank (cols 0:128 chunk0, 128:256 chunk1, 256:384 denom);
  ACT flushes it to SBUF at group end, then it's DMA'd out immediately.

ow is built per 16-tile supertile in 2 wide DVE ops (+1 ACT op):
  mask[n,j,w] = (yg[n,j] == iota[w])        one TT is_equal w/ broadcast APs
  prep[n,j,h,w] = Exp(slr[n,j,h])           ACT exp, stride-0 bcast over w
  ow = mask (bcast over h) * prep           one TT mult
which sidesteps the ~190ns fixed cost of narrow per-tile DVE ops.

DMA: host pre-lays-out everything contiguous; hn carries yg as column 256;
h^T goes on the ACT HWDGE ring, hn on the SP ring; constants are one DMA;
each group's result is DMA'd out as soon as it's flushed.
"""

import sys

sys.path.insert(0, "/opt/trn_rl_repo")

import numpy as np
import ml_dtypes

BF = ml_dtypes.bfloat16

N = 100000
INC = 256
NHEAD = 4
OUTC = 128
NCLS = 1000
NCORES = 8
CPC = NCLS // NCORES  # 125 classes per core
GCLS = 32  # classes per group window
NGRP = 4  # groups per core (32+32+32+29)
NCOL = INC + 2  # hn row: 256 features, yg, zero pad
BANKW = 512  # one PSUM bank; 0:128 c0, 128:256 c1, 256:384 denom

_PROG_CACHE = {}
LAST_RESULT = None
LAST_PROFILE = None


def build_program(tg, s_tiles=16):
    """Build + compile the SPMD Bass program. tg = tiles per group (len 4)."""
    from concourse import bacc, mybir, tile

    f32 = mybir.dt.float32
    bf16 = mybir.dt.bfloat16
    AF = mybir.ActivationFunctionType
    OP = mybir.AluOpType

    t_tiles = int(sum(tg))
    cap = t_tiles * 128
    grp_of = []
    for g, tcnt in enumerate(tg):
        for i in range(tcnt):
            grp_of.append((g, i == 0, i == tcnt - 1))

    nc = bacc.Bacc(
        "TRN2", target_bir_lowering=False, debug=False, num_devices=NCORES
    )

    hn = nc.dram_tensor("hn", [128, t_tiles, NCOL], bf16, kind="ExternalInput").ap()
    htr = nc.dram_tensor("htr", [128, 2, cap], bf16, kind="ExternalInput").ap()
    # consts: cols 0:32 iota, 32:36 V chunk0, 36:40 V chunk1, row0 40:44 c
    cst = nc.dram_tensor("cst", [128, 44], bf16, kind="ExternalInput").ap()
    out = nc.dram_tensor(
        "outp", [128, NGRP * 384], f32, kind="ExternalOutput"
    ).ap()

    nsup = (t_tiles + s_tiles - 1) // s_tiles

    def sup_range(s_):
        ts0 = s_ * s_tiles
        return ts0, min(s_tiles, t_tiles - ts0)

    with tile.TileContext(nc) as tc:
        with (
            tc.tile_pool(name="const", bufs=1) as cpool,
            tc.tile_pool(name="stream", bufs=4) as sb,
            tc.tile_pool(name="work", bufs=3) as sg,
            tc.tile_pool(name="ps", bufs=4, space="PSUM") as ps,
            tc.tile_pool(name="acc", bufs=2, space="PSUM") as accp,
        ):
            # ---- constants (one DMA) --------------------------------------
            cst_sb = cpool.tile([128, 44], bf16)
            nc.sync.dma_start(out=cst_sb[:], in_=cst)
            ciota_sb = cst_sb[:, 0:GCLS]
            v_bf = [cst_sb[:, 32:36], cst_sb[:, 36:40]]
            ones_row = cpool.tile([1, 128], bf16)
            nc.vector.memset(ones_row[:1], 1.0)
            ones_col = cpool.tile([128, 1], bf16)
            nc.vector.memset(ones_col[:], 1.0)
            c_rep = cpool.tile([1, s_tiles * NHEAD], bf16)
            nc.vector.tensor_copy(
                out=c_rep[:1].rearrange("p (j h) -> p j h", h=NHEAD),
                in_=cst_sb[0:1, 40:44]
                .to_broadcast([1, NHEAD, s_tiles])
                .rearrange("p h j -> p j h"),
            )

            out_sb = cpool.tile([128, NGRP * 384], f32)

            stream_tiles = {}
            ow_tiles = {}
            acc_state = {"t": None}

            def load_and_scores(s_):
                ts0, nst = sup_range(s_)
                c0 = ts0 * 128
                c1 = c0 + nst * 128
                nh1 = (nst + 1) // 2  # first-half tiles
                cm = c0 + nh1 * 128
                # h^T chunks on the ACT HWDGE ring (two halves so the first
                # score matmuls start sooner); node-major on the SP ring
                ht = sb.tile([128, 2, s_tiles * 128], bf16, tag="ht")
                nc.scalar.dma_start(
                    out=ht[:, :, : nh1 * 128], in_=htr[:, :, c0:cm]
                )
                if nst > nh1:
                    nc.scalar.dma_start(
                        out=ht[:, :, nh1 * 128 : nst * 128], in_=htr[:, :, cm:c1]
                    )
                hns = sb.tile([128, s_tiles, NCOL], bf16, tag="hn")
                nc.sync.dma_start(out=hns[:, :nst, :], in_=hn[:, ts0 : ts0 + nst, :])
                stream_tiles[s_] = (ht, hns, nst)

                # mask is off the score critical path (needs only the DMA)
                mask = sg.tile([128, s_tiles, GCLS], bf16, tag="mask")
                nc.vector.tensor_tensor(
                    out=mask[:, :nst, :],
                    in0=hns[:, :nst, INC].to_broadcast([128, nst, GCLS]),
                    in1=ciota_sb.to_broadcast([128, GCLS, nst]).rearrange(
                        "p w j -> p j w"
                    ),
                    op=OP.is_equal,
                )

                # scores for the whole supertile into one PSUM bank
                s_ps = ps.tile([128, s_tiles * NHEAD], f32, tag="sps")
                nw = nst * NHEAD
                nc.tensor.matmul(
                    s_ps[:, :nw], lhsT=ones_row[:1], rhs=c_rep[:1, :nw],
                    start=True, stop=False, skip_group_check=True,
                )
                for j in range(nst):
                    sl = slice(j * NHEAD, (j + 1) * NHEAD)
                    nc.tensor.matmul(
                        s_ps[:, sl], lhsT=ht[:, 0, j * 128 : (j + 1) * 128],
                        rhs=v_bf[0], start=False, stop=False,
                        skip_group_check=True,
                    )
                    nc.tensor.matmul(
                        s_ps[:, sl], lhsT=ht[:, 1, j * 128 : (j + 1) * 128],
                        rhs=v_bf[1], start=False, stop=True,
                        skip_group_check=True,
                    )
                # p = exp(leaky_relu(s, 0.2)) in two half-supertile slices so
                # the chain starts after only half the score matmuls; the exp
                # broadcasts p across the 32-class window (stride-0 read).
                t02 = sg.tile([128, s_tiles * NHEAD], f32, tag="t02")
                slr = sg.tile([128, s_tiles * NHEAD], f32, tag="slr")
                prep = sg.tile([128, s_tiles, NHEAD, GCLS], bf16, tag="prep")
                ow = sb.tile([128, s_tiles, NHEAD, GCLS], bf16, tag="ow")
                for j0, j1 in ((0, nh1), (nh1, nst)):
                    if j1 <= j0:
                        continue
                    w0, w1 = j0 * NHEAD, j1 * NHEAD
                    nc.vector.tensor_scalar_mul(
                        t02[:, w0:w1], s_ps[:, w0:w1], 0.2
                    )
                    nc.vector.tensor_tensor(
                        out=slr[:, w0:w1], in0=s_ps[:, w0:w1],
                        in1=t02[:, w0:w1], op=OP.max,
                    )
                    nc.scalar.activation(
                        prep[:, j0:j1],
                        slr[:, w0:w1]
                        .rearrange("p (j h) -> p j h", h=NHEAD)
                        .to_broadcast([128, j1 - j0, NHEAD, GCLS]),
                        AF.Exp,
                    )
                    nc.vector.tensor_tensor(
                        out=ow[:, j0:j1],
                        in0=mask[:, j0:j1, :]
                        .to_broadcast([128, j1 - j0, GCLS, NHEAD])
                        .rearrange("p j w h -> p j h w"),
                        in1=prep[:, j0:j1],
                        op=OP.mult,
                    )
                ow_tiles[s_] = ow

            def pools(s_):
                ht, hns, nst = stream_tiles.pop(s_)
                ow = ow_tiles.pop(s_)
                ts0, _ = sup_range(s_)
                for j in range(nst):
                    t_ = ts0 + j
                    g, first, last = grp_of[t_]
                    if first:
                        acc_state["t"] = accp.tile(
                            [128, BANKW], f32, tag="gacc", name="gacc"
                        )
                    acc = acc_state["t"]
                    rhs = ow[:, j]
                    nc.tensor.matmul(
                        acc[:, 0:128], lhsT=hns[:, j, 0:128], rhs=rhs,
                        start=first, stop=last, skip_group_check=True,
                    )
                    nc.tensor.matmul(
                        acc[:, 128:256], lhsT=hns[:, j, 128:256], rhs=rhs,
                        start=False, stop=last, skip_group_check=True,
                    )
                    nc.tensor.matmul(
                        acc[0:1, 256:384], lhsT=ones_col[:], rhs=rhs,
                        start=False, stop=last, skip_group_check=True,
                    )
                    if last:
                        nc.scalar.activation(
                            out_sb[:, g * 384 : (g + 1) * 384],
                            acc[:, 0:384], AF.Copy,
                        )
                        nc.sync.dma_start(
                            out=out[:, g * 384 : (g + 1) * 384],
                            in_=out_sb[:, g * 384 : (g + 1) * 384],
                        )

            for s_ in range(min(3, nsup)):
                load_and_scores(s_)
            for s_ in range(nsup):
                if s_ + 3 < nsup:
                    load_and_scores(s_ + 3)
                pools(s_)

    nc.compile()
    return nc


def _prep_inputs(context_h, W_lin, b_lin, W_att, b_att, context_y):
    """Host-side shard: argsort by class, 8 class ranges of 125, each split
    into 4 groups of <=32 classes, nodes padded to whole 128-node tiles."""
    h = np.asarray(context_h, dtype=np.float32)
    hb = h.astype(BF)
    y = np.asarray(context_y).astype(np.int64)
    order = np.argsort(y, kind="stable")
    ys = y[order]

    gb = []
    for k in range(NCORES):
        for g in range(NGRP):
            gb.append(k * CPC + min(g * GCLS, CPC))
    gb.append(NCLS)
    gbounds = np.searchsorted(ys, np.asarray(gb))
    cnts = (gbounds[1:] - gbounds[:-1]).reshape(NCORES, NGRP)
    tg = tuple(int(-(-int(cnts[:, g].max()) // 128)) for g in range(NGRP))
    t_tiles = int(sum(tg))

    # fold V[k,h], c[h] from W_lin/W_att/b_lin/b_att (weights-only preproc)
    W_lin = np.asarray(W_lin, dtype=np.float32)
    W_att = np.asarray(W_att, dtype=np.float32)
    V = np.einsum("hok,o->kh", W_lin.reshape(NHEAD, OUTC, INC), W_att)  # [256,4]
    c = (
        np.asarray(b_lin, dtype=np.float32).reshape(NHEAD, OUTC) @ W_att
        + np.asarray(b_att, dtype=np.float32)[0]
    )  # [4]
    cst = np.zeros((128, 44), dtype=BF)
    cst[:, 0:GCLS] = np.arange(GCLS, dtype=np.float32)[None, :].astype(BF)
    cst[:, 32:36] = V[0:128].astype(BF)
    cst[:, 36:40] = V[128:256].astype(BF)
    cst[0, 40:44] = c.astype(BF)

    in_maps = []
    for k in range(NCORES):
        hp = np.zeros((t_tiles * 128, NCOL), dtype=BF)
        hp[:, INC] = BF(float(GCLS + 95))  # pad marker 127 -> mask==0
        row = 0
        for g in range(NGRP):
            gi = k * NGRP + g
            lo, hi = gbounds[gi], gbounds[gi + 1]
            cnt = hi - lo
            idx = order[lo:hi]
            hp[row : row + cnt, 0:INC] = hb[idx]
            hp[row : row + cnt, INC] = (
                ys[lo:hi] - (k * CPC + g * GCLS)
            ).astype(np.float32).astype(BF)
            row += tg[g] * 128
        hn = np.ascontiguousarray(
            hp.reshape(t_tiles, 128, NCOL).transpose(1, 0, 2)
        )
        htr = np.ascontiguousarray(
            hp[:, 0:INC].reshape(t_tiles * 128, 2, 128).transpose(2, 1, 0)
        )
        in_maps.append({"hn": hn, "htr": htr, "cst": cst})
    return in_maps, tg


def kernel(context_h, W_lin, b_lin, W_att, b_att, context_y, num_classes):
    global LAST_RESULT, LAST_PROFILE
    import os

    assert int(num_classes) == NCLS

    from concourse.bass_utils import run_bass_kernel_spmd

    in_maps, tg = _prep_inputs(
        context_h, W_lin, b_lin, W_att, b_att, context_y
    )
    if tg not in _PROG_CACHE:
        _PROG_CACHE[tg] = build_program(tg)
    nc = _PROG_CACHE[tg]
    core_ids = list(range(NCORES))
    res = run_bass_kernel_spmd(nc, in_maps, core_ids)
    LAST_RESULT = res

    if os.environ.get("KERNEL_PROFILE") == "1":
        LAST_PROFILE = run_bass_kernel_spmd(nc, in_maps, core_ids, trace=True)

    outp = np.empty((NCLS, NHEAD * INC), dtype=np.float32)
    for k in range(NCORES):
        o = np.asarray(res.results[k]["outp"])  # [128, 4*384]
        for g in range(NGRP):
            blk = o[:, g * 384 : (g + 1) * 384]
            ncls_g = min(GCLS, CPC - g * GCLS)
            c0 = blk[:, 0:128].reshape(128, NHEAD, GCLS)
            c1 = blk[:, 128:256].reshape(128, NHEAD, GCLS)
            den = blk[0, 256:384].reshape(NHEAD, GCLS)
            den = np.where(den != 0.0, den, 1.0)
            pooled = np.concatenate([c0, c1], axis=0)  # [256, h, w]
            pooled = pooled / den[None, :, :]
            pw = pooled.transpose(2, 1, 0).reshape(GCLS, NHEAD * INC)
            base = k * CPC + g * GCLS
            outp[base : base + ncls_g] = pw[:ncls_g]
    return outp


# revision 13
# speedup vs baseline: 1.8450x; 1.1416x over previous
"""Trainium2 Bass kernel for nn_AttentionPool (gnn_message_passing).

Strategy (v4: class-group windowed pooling, supertile-batched one-hot)
----------------------------------------------------------------------
Math restructure (exactly equivalent to the reference up to fp rounding):
  score[n,h] = context_h[n,:] @ V[:,h] + c[h]        (V, c host-folded from
               W_lin/W_att/b_lin/b_att: V[k,h] = sum_o W_lin[h*128+o,k]W_att[o],
               c[h] = b_lin[h*128:(h+1)*128].W_att + b_att)
  p = exp(leaky_relu(score, 0.2))                     (skip segment-max: scores
               are O(1) so exp cannot overflow; softmax is shift-invariant)
  denom[cls,h]  = sum_{n: y=cls} p[n,h]
  pooled[cls,h,:] = sum_{n: y=cls} p[n,h]*context_h[n,:] / denom[cls,h]

Sharding: BY CLASS. Host argsorts context_y; core k owns classes
[125k, 125k+125) -> no cross-core reduction.

Within a core, classes split into 4 GROUPS of <=32 consecutive classes; the
sorted nodes of each group are padded to whole 128-node tiles, so every tile
touches one 32-class window and the scatter-add matmul runs "flipped":

  per tile j (128 nodes) in group g:
    ow[n, h*32+w] = (yg[n]==w) * p[n,h]
    acc_g[c,    hw] += hn_j[:, 0:128].T @ ow      (feat chunk 0)
    acc_g[c+.., hw] += hn_j[:,128:256].T @ ow     (feat chunk 1)
    acc_g[0(d), hw] += ones.T         @ ow        (denom row)
  acc_g = one PSUM bank (cols 0:128 chunk0, 128:256 chunk1, 256:384 denom);
  ACT flushes it to SBUF at group end, then it's DMA'd out immediately.

ow is built per 16-tile supertile in 2 wide DVE ops (+1 ACT op):
  mask[n,j,w] = (yg[n,j] == iota[w])        one TT is_equal w/ broadcast APs
  prep[n,j,h,w] = Exp(slr[n,j,h])           ACT exp, stride-0 bcast over w
  ow = mask (bcast over h) * prep           one TT mult
which sidesteps the ~190ns fixed cost of narrow per-tile DVE ops.

DMA: host pre-lays-out everything contiguous; hn carries yg as column 256;
h^T goes on the ACT HWDGE ring, hn on the SP ring; constants are one DMA;
each group's result is DMA'd out as soon as it's flushed.
"""

import sys

sys.path.insert(0, "/opt/trn_rl_repo")

import numpy as np
import ml_dtypes

BF = ml_dtypes.bfloat16

N = 100000
INC = 256
NHEAD = 4
OUTC = 128
NCLS = 1000
NCORES = 8
CPC = NCLS // NCORES  # 125 classes per core
GCLS = 32  # classes per group window
NGRP = 4  # groups per core (32+32+32+29)
NCOL = INC + 2  # hn row: 256 features, yg, zero pad
BANKW = 512  # one PSUM bank; 0:128 c0, 128:256 c1, 256:384 denom

_PROG_CACHE = {}
LAST_RESULT = None
LAST_PROFILE = None


def build_program(tg, s_tiles=16):
    """Build + compile the SPMD Bass program. tg = tiles per group (len 4)."""
    from concourse import bacc, mybir, tile

    f32 = mybir.dt.float32
    bf16 = mybir.dt.bfloat16
    AF = mybir.ActivationFunctionType
    OP = mybir.AluOpType

    t_tiles = int(sum(tg))
    cap = t_tiles * 128
    grp_of = []
    for g, tcnt in enumerate(tg):
        for i in range(tcnt):
            grp_of.append((g, i == 0, i == tcnt - 1))

    nc = bacc.Bacc(
        "TRN2", target_bir_lowering=False, debug=False, num_devices=NCORES
    )

    hn = nc.dram_tensor("hn", [128, t_tiles, NCOL], bf16, kind="ExternalInput").ap()
    htr = nc.dram_tensor("htr", [128, 2, cap], bf16, kind="ExternalInput").ap()
    # consts: cols 0:32 iota, 32:36 V chunk0, 36:40 V chunk1, row0 40:44 c
    cst = nc.dram_tensor("cst", [128, 44], bf16, kind="ExternalInput").ap()
    out = nc.dram_tensor(
        "outp", [128, NGRP * 384], f32, kind="ExternalOutput"
    ).ap()

    # supertile schedule: small ones first so the pipeline primes while the
    # first DMAs stream, then full 16-tile supertiles
    sizes = []
    rem = t_tiles
    for sz in (4, 8):
        if rem > sz + s_tiles:
            sizes.append(sz)
            rem -= sz
    while rem > 0:
        sizes.append(min(s_tiles, rem))
        rem -= sizes[-1]
    starts = []
    acc0 = 0
    for sz in sizes:
        starts.append(acc0)
        acc0 += sz
    nsup = len(sizes)

    def sup_range(s_):
        return starts[s_], sizes[s_]

    with tile.TileContext(nc) as tc:
        with (
            tc.tile_pool(name="const", bufs=1) as cpool,
            tc.tile_pool(name="stream", bufs=4) as sb,
            tc.tile_pool(name="work", bufs=3) as sg,
            tc.tile_pool(name="ps", bufs=4, space="PSUM") as ps,
            tc.tile_pool(name="acc", bufs=2, space="PSUM") as accp,
        ):
            # ---- constants (one DMA) --------------------------------------
            cst_sb = cpool.tile([128, 44], bf16)
            nc.sync.dma_start(out=cst_sb[:], in_=cst)
            ciota_sb = cst_sb[:, 0:GCLS]
            v_bf = [cst_sb[:, 32:36], cst_sb[:, 36:40]]
            ones_row = cpool.tile([1, 128], bf16)
            nc.vector.memset(ones_row[:1], 1.0)
            ones_col = cpool.tile([128, 1], bf16)
            nc.vector.memset(ones_col[:], 1.0)
            c_rep = cpool.tile([1, s_tiles * NHEAD], bf16)
            nc.vector.tensor_copy(
                out=c_rep[:1].rearrange("p (j h) -> p j h", h=NHEAD),
                in_=cst_sb[0:1, 40:44]
                .to_broadcast([1, NHEAD, s_tiles])
                .rearrange("p h j -> p j h"),
            )

            out_sb = cpool.tile([128, NGRP * 384], f32)

            stream_tiles = {}
            ow_tiles = {}
            acc_state = {"t": None}

            def load_and_scores(s_):
                ts0, nst = sup_range(s_)
                c0 = ts0 * 128
                c1 = c0 + nst * 128
                nh1 = (nst + 1) // 2  # first-half tiles
                cm = c0 + nh1 * 128
                # h^T chunks on the ACT HWDGE ring (two halves so the first
                # score matmuls start sooner); node-major on the SP ring
                ht = sb.tile([128, 2, s_tiles * 128], bf16, tag="ht")
                nc.scalar.dma_start(
                    out=ht[:, :, : nh1 * 128], in_=htr[:, :, c0:cm]
                )
                if nst > nh1:
                    nc.scalar.dma_start(
                        out=ht[:, :, nh1 * 128 : nst * 128], in_=htr[:, :, cm:c1]
                    )
                hns = sb.tile([128, s_tiles, NCOL], bf16, tag="hn")
                nc.sync.dma_start(out=hns[:, :nst, :], in_=hn[:, ts0 : ts0 + nst, :])
                stream_tiles[s_] = (ht, hns, nst)

                # mask is off the score critical path (needs only the DMA)
                mask = sg.tile([128, s_tiles, GCLS], bf16, tag="mask")
                nc.vector.tensor_tensor(
                    out=mask[:, :nst, :],
                    in0=hns[:, :nst, INC].to_broadcast([128, nst, GCLS]),
                    in1=ciota_sb.to_broadcast([128, GCLS, nst]).rearrange(
                        "p w j -> p j w"
                    ),
                    op=OP.is_equal,
                )

                # scores for the whole supertile into one PSUM bank
                s_ps = ps.tile([128, s_tiles * NHEAD], f32, tag="sps")
                nw = nst * NHEAD
                nc.tensor.matmul(
                    s_ps[:, :nw], lhsT=ones_row[:1], rhs=c_rep[:1, :nw],
                    start=True, stop=False, skip_group_check=True,
                )
                for j in range(nst):
                    sl = slice(j * NHEAD, (j + 1) * NHEAD)
                    nc.tensor.matmul(
                        s_ps[:, sl], lhsT=ht[:, 0, j * 128 : (j + 1) * 128],
                        rhs=v_bf[0], start=False, stop=False,
                        skip_group_check=True,
                    )
                    nc.tensor.matmul(
                        s_ps[:, sl], lhsT=ht[:, 1, j * 128 : (j + 1) * 128],
                        rhs=v_bf[1], start=False, stop=True,
                        skip_group_check=True,
                    )
                # p = exp(leaky_relu(s, 0.2)) in two half-supertile slices so
                # the chain starts after only half the score matmuls; the exp
                # broadcasts p across the 32-class window (stride-0 read).
                t02 = sg.tile([128, s_tiles * NHEAD], f32, tag="t02")
                slr = sg.tile([128, s_tiles * NHEAD], f32, tag="slr")
                prep = sg.tile([128, s_tiles, NHEAD, GCLS], bf16, tag="prep")
                ow = sb.tile([128, s_tiles, NHEAD, GCLS], bf16, tag="ow")
                for j0, j1 in ((0, nh1), (nh1, nst)):
                    if j1 <= j0:
                        continue
                    w0, w1 = j0 * NHEAD, j1 * NHEAD
                    nc.vector.tensor_scalar_mul(
                        t02[:, w0:w1], s_ps[:, w0:w1], 0.2
                    )
                    nc.vector.tensor_tensor(
                        out=slr[:, w0:w1], in0=s_ps[:, w0:w1],
                        in1=t02[:, w0:w1], op=OP.max,
                    )
                    nc.scalar.activation(
                        prep[:, j0:j1],
                        slr[:, w0:w1]
                        .rearrange("p (j h) -> p j h", h=NHEAD)
                        .to_broadcast([128, j1 - j0, NHEAD, GCLS]),
                        AF.Exp,
                    )
                    nc.vector.tensor_tensor(
                        out=ow[:, j0:j1],
                        in0=mask[:, j0:j1, :]
                        .to_broadcast([128, j1 - j0, GCLS, NHEAD])
                        .rearrange("p j w h -> p j h w"),
                        in1=prep[:, j0:j1],
                        op=OP.mult,
                    )
                ow_tiles[s_] = ow

            def pools(s_):
                ht, hns, nst = stream_tiles.pop(s_)
                ow = ow_tiles.pop(s_)
                ts0, _ = sup_range(s_)
                accs = []
                ended = []
                for j in range(nst):
                    t_ = ts0 + j
                    g, first, last = grp_of[t_]
                    if first:
                        acc_state["t"] = accp.tile(
                            [128, BANKW], f32, tag="gacc", name="gacc"
                        )
                    acc = acc_state["t"]
                    accs.append(acc)
                    rhs = ow[:, j]
                    nc.tensor.matmul(
                        acc[:, 0:128], lhsT=hns[:, j, 0:128], rhs=rhs,
                        start=first, stop=last, skip_group_check=True,
                    )
                    nc.tensor.matmul(
                        acc[:, 128:256], lhsT=hns[:, j, 128:256], rhs=rhs,
                        start=False, stop=last, skip_group_check=True,
                    )
                    if last:
                        ended.append((g, acc))
                # hoisted denom block: ones stays the stationary operand for
                # nst back-to-back matmuls (one weight load)
                for j in range(nst):
                    t_ = ts0 + j
                    g, first, last = grp_of[t_]
                    nc.tensor.matmul(
                        accs[j][0:1, 256:384], lhsT=ones_col[:], rhs=ow[:, j],
                        start=False, stop=last, skip_group_check=True,
                    )
                for g, acc in ended:
                    nc.scalar.activation(
                        out_sb[:, g * 384 : (g + 1) * 384],
                        acc[:, 0:384], AF.Copy,
                    )
                    nc.sync.dma_start(
                        out=out[:, g * 384 : (g + 1) * 384],
                        in_=out_sb[:, g * 384 : (g + 1) * 384],
                    )

            for s_ in range(min(3, nsup)):
                load_and_scores(s_)
            for s_ in range(nsup):
                if s_ + 3 < nsup:
                    load_and_scores(s_ + 3)
                pools(s_)

    nc.compile()
    return nc


def _prep_inputs(context_h, W_lin, b_lin, W_att, b_att, context_y):
    """Host-side shard: argsort by class, 8 class ranges of 125, each split
    into 4 groups of <=32 classes, nodes padded to whole 128-node tiles."""
    h = np.asarray(context_h, dtype=np.float32)
    hb = h.astype(BF)
    y = np.asarray(context_y).astype(np.int64)
    order = np.argsort(y, kind="stable")
    ys = y[order]

    gb = []
    for k in range(NCORES):
        for g in range(NGRP):
            gb.append(k * CPC + min(g * GCLS, CPC))
    gb.append(NCLS)
    gbounds = np.searchsorted(ys, np.asarray(gb))
    cnts = (gbounds[1:] - gbounds[:-1]).reshape(NCORES, NGRP)
    tg = tuple(int(-(-int(cnts[:, g].max()) // 128)) for g in range(NGRP))
    t_tiles = int(sum(tg))

    # fold V[k,h], c[h] from W_lin/W_att/b_lin/b_att (weights-only preproc)
    W_lin = np.asarray(W_lin, dtype=np.float32)
    W_att = np.asarray(W_att, dtype=np.float32)
    V = np.einsum("hok,o->kh", W_lin.reshape(NHEAD, OUTC, INC), W_att)  # [256,4]
    c = (
        np.asarray(b_lin, dtype=np.float32).reshape(NHEAD, OUTC) @ W_att
        + np.asarray(b_att, dtype=np.float32)[0]
    )  # [4]
    cst = np.zeros((128, 44), dtype=BF)
    cst[:, 0:GCLS] = np.arange(GCLS, dtype=np.float32)[None, :].astype(BF)
    cst[:, 32:36] = V[0:128].astype(BF)
    cst[:, 36:40] = V[128:256].astype(BF)
    cst[0, 40:44] = c.astype(BF)

    in_maps = []
    for k in range(NCORES):
        hp = np.zeros((t_tiles * 128, NCOL), dtype=BF)
        hp[:, INC] = BF(float(GCLS + 95))  # pad marker 127 -> mask==0
        row = 0
        for g in range(NGRP):
            gi = k * NGRP + g
            lo, hi = gbounds[gi], gbounds[gi + 1]
            cnt = hi - lo
            idx = order[lo:hi]
            hp[row : row + cnt, 0:INC] = hb[idx]
            hp[row : row + cnt, INC] = (
                ys[lo:hi] - (k * CPC + g * GCLS)
            ).astype(np.float32).astype(BF)
            row += tg[g] * 128
        hn = np.ascontiguousarray(
            hp.reshape(t_tiles, 128, NCOL).transpose(1, 0, 2)
        )
        htr = np.ascontiguousarray(
            hp[:, 0:INC].reshape(t_tiles * 128, 2, 128).transpose(2, 1, 0)
        )
        in_maps.append({"hn": hn, "htr": htr, "cst": cst})
    return in_maps, tg


def kernel(context_h, W_lin, b_lin, W_att, b_att, context_y, num_classes):
    global LAST_RESULT, LAST_PROFILE
    import os

    assert int(num_classes) == NCLS

    from concourse.bass_utils import run_bass_kernel_spmd

    in_maps, tg = _prep_inputs(
        context_h, W_lin, b_lin, W_att, b_att, context_y
    )
    if tg not in _PROG_CACHE:
        _PROG_CACHE[tg] = build_program(tg)
    nc = _PROG_CACHE[tg]
    core_ids = list(range(NCORES))
    res = run_bass_kernel_spmd(nc, in_maps, core_ids)
    LAST_RESULT = res

    if os.environ.get("KERNEL_PROFILE") == "1":
        LAST_PROFILE = run_bass_kernel_spmd(nc, in_maps, core_ids, trace=True)

    outp = np.empty((NCLS, NHEAD * INC), dtype=np.float32)
    for k in range(NCORES):
        o = np.asarray(res.results[k]["outp"])  # [128, 4*384]
        for g in range(NGRP):
            blk = o[:, g * 384 : (g + 1) * 384]
            ncls_g = min(GCLS, CPC - g * GCLS)
            c0 = blk[:, 0:128].reshape(128, NHEAD, GCLS)
            c1 = blk[:, 128:256].reshape(128, NHEAD, GCLS)
            den = blk[0, 256:384].reshape(NHEAD, GCLS)
            den = np.where(den != 0.0, den, 1.0)
            pooled = np.concatenate([c0, c1], axis=0)  # [256, h, w]
            pooled = pooled / den[None, :, :]
            pw = pooled.transpose(2, 1, 0).reshape(GCLS, NHEAD * INC)
            base = k * CPC + g * GCLS
            outp[base : base + ncls_g] = pw[:ncls_g]
    return outp


# revision 16
# speedup vs baseline: 1.9461x; 1.0548x over previous
"""Trainium2 Bass kernel for nn_AttentionPool (gnn_message_passing).

Strategy (v4: class-group windowed pooling, supertile-batched one-hot)
----------------------------------------------------------------------
Math restructure (exactly equivalent to the reference up to fp rounding):
  score[n,h] = context_h[n,:] @ V[:,h] + c[h]        (V, c host-folded from
               W_lin/W_att/b_lin/b_att: V[k,h] = sum_o W_lin[h*128+o,k]W_att[o],
               c[h] = b_lin[h*128:(h+1)*128].W_att + b_att)
  p = exp(leaky_relu(score, 0.2))                     (skip segment-max: scores
               are O(1) so exp cannot overflow; softmax is shift-invariant)
  denom[cls,h]  = sum_{n: y=cls} p[n,h]
  pooled[cls,h,:] = sum_{n: y=cls} p[n,h]*context_h[n,:] / denom[cls,h]

Sharding: BY CLASS. Host argsorts context_y; core k owns classes
[125k, 125k+125) -> no cross-core reduction.

Within a core, classes split into 4 GROUPS of <=32 consecutive classes; the
sorted nodes of each group are padded to whole 128-node tiles, so every tile
touches one 32-class window and the scatter-add matmul runs "flipped":

  per tile j (128 nodes) in group g:
    ow[n, h*32+w] = (yg[n]==w) * p[n,h]
    acc_g[c,    hw] += hn_j[:, 0:128].T @ ow      (feat chunk 0)
    acc_g[c+.., hw] += hn_j[:,128:256].T @ ow     (feat chunk 1)
    acc_g[0(d), hw] += ones.T         @ ow        (denom row)
  acc_g = one PSUM bank (cols 0:128 chunk0, 128:256 chunk1, 256:384 denom);
  ACT flushes it to SBUF at group end, then it's DMA'd out immediately.

ow is built per 16-tile supertile in 2 wide DVE ops (+1 ACT op):
  mask[n,j,w] = (yg[n,j] == iota[w])        one TT is_equal w/ broadcast APs
  prep[n,j,h,w] = Exp(slr[n,j,h])           ACT exp, stride-0 bcast over w
  ow = mask (bcast over h) * prep           one TT mult
which sidesteps the ~190ns fixed cost of narrow per-tile DVE ops.

DMA: host pre-lays-out everything contiguous; hn carries yg as column 256;
h^T goes on the ACT HWDGE ring, hn on the SP ring; constants are one DMA;
each group's result is DMA'd out as soon as it's flushed.
"""

import sys

sys.path.insert(0, "/opt/trn_rl_repo")

import numpy as np
import ml_dtypes

BF = ml_dtypes.bfloat16

N = 100000
INC = 256
NHEAD = 4
OUTC = 128
NCLS = 1000
NCORES = 8
CPC = NCLS // NCORES  # 125 classes per core
GCLS = 32  # classes per group window
NGRP = 4  # groups per core (32+32+32+29)
NCOL = INC + 2  # hn row: 256 features, yg, zero pad
BANKW = 512  # one PSUM bank; 0:128 c0, 128:256 c1, 256:384 denom

_PROG_CACHE = {}
LAST_RESULT = None
LAST_PROFILE = None


def build_program(tg, s_tiles=16):
    """Build + compile the SPMD Bass program. tg = tiles per group (len 4)."""
    from concourse import bacc, mybir, tile

    f32 = mybir.dt.float32
    bf16 = mybir.dt.bfloat16
    AF = mybir.ActivationFunctionType
    OP = mybir.AluOpType

    t_tiles = int(sum(tg))
    cap = t_tiles * 128
    grp_of = []
    for g, tcnt in enumerate(tg):
        for i in range(tcnt):
            grp_of.append((g, i == 0, i == tcnt - 1))

    nc = bacc.Bacc(
        "TRN2", target_bir_lowering=False, debug=False, num_devices=NCORES
    )

    hn = nc.dram_tensor("hn", [128, t_tiles, NCOL], bf16, kind="ExternalInput").ap()
    htr = nc.dram_tensor("htr", [128, 2, cap], bf16, kind="ExternalInput").ap()
    # consts: cols 0:32 iota, 32:36 V chunk0, 36:40 V chunk1, row0 40:44 c
    cst = nc.dram_tensor("cst", [128, 44], bf16, kind="ExternalInput").ap()
    out = nc.dram_tensor(
        "outp", [128, NGRP * 384], f32, kind="ExternalOutput"
    ).ap()

    # supertile schedule: small ones first so the pipeline primes while the
    # first DMAs stream, then full 16-tile supertiles
    sizes = []
    rem = t_tiles
    for sz in (2, 4, 8):
        if rem > sz + s_tiles:
            sizes.append(sz)
            rem -= sz
    while rem > 0:
        sizes.append(min(s_tiles, rem))
        rem -= sizes[-1]
    starts = []
    acc0 = 0
    for sz in sizes:
        starts.append(acc0)
        acc0 += sz
    nsup = len(sizes)

    def sup_range(s_):
        return starts[s_], sizes[s_]

    with tile.TileContext(nc) as tc:
        with (
            tc.tile_pool(name="const", bufs=1) as cpool,
            tc.tile_pool(name="stream", bufs=4) as sb,
            tc.tile_pool(name="work", bufs=3) as sg,
            tc.tile_pool(name="ps", bufs=4, space="PSUM") as ps,
            tc.tile_pool(name="acc", bufs=2, space="PSUM") as accp,
        ):
            # ---- constants (one DMA) --------------------------------------
            cst_sb = cpool.tile([128, 44], bf16)
            nc.sync.dma_start(out=cst_sb[:], in_=cst)
            ciota_sb = cst_sb[:, 0:GCLS]
            v_bf = [cst_sb[:, 32:36], cst_sb[:, 36:40]]
            ones_row = cpool.tile([1, 128], bf16)
            nc.vector.memset(ones_row[:1], 1.0)
            ones_col = cpool.tile([128, 1], bf16)
            nc.vector.memset(ones_col[:], 1.0)
            c_rep = cpool.tile([1, s_tiles * NHEAD], bf16)
            nc.vector.tensor_copy(
                out=c_rep[:1].rearrange("p (j h) -> p j h", h=NHEAD),
                in_=cst_sb[0:1, 40:44]
                .to_broadcast([1, NHEAD, s_tiles])
                .rearrange("p h j -> p j h"),
            )

            out_sb = cpool.tile([128, NGRP * 384], f32)

            stream_tiles = {}
            ow_tiles = {}
            acc_state = {"t": None}

            def load_and_scores(s_):
                ts0, nst = sup_range(s_)
                c0 = ts0 * 128
                c1 = c0 + nst * 128
                nh1 = (nst + 1) // 2  # first-half tiles
                # h^T chunks on the ACT HWDGE ring; node-major on the SP ring
                ht = sb.tile([128, 2, s_tiles * 128], bf16, tag="ht")
                nc.scalar.dma_start(
                    out=ht[:, :, : nst * 128], in_=htr[:, :, c0:c1]
                )
                hns = sb.tile([128, s_tiles, NCOL], bf16, tag="hn")
                nc.sync.dma_start(out=hns[:, :nst, :], in_=hn[:, ts0 : ts0 + nst, :])
                stream_tiles[s_] = (ht, hns, nst)

                # mask is off the score critical path (needs only the DMA)
                mask = sg.tile([128, s_tiles, GCLS], bf16, tag="mask")
                nc.vector.tensor_tensor(
                    out=mask[:, :nst, :],
                    in0=hns[:, :nst, INC].to_broadcast([128, nst, GCLS]),
                    in1=ciota_sb.to_broadcast([128, GCLS, nst]).rearrange(
                        "p w j -> p j w"
                    ),
                    op=OP.is_equal,
                )

                # scores for the whole supertile into one PSUM bank
                s_ps = ps.tile([128, s_tiles * NHEAD], f32, tag="sps")
                nw = nst * NHEAD
                nc.tensor.matmul(
                    s_ps[:, :nw], lhsT=ones_row[:1], rhs=c_rep[:1, :nw],
                    start=True, stop=False, skip_group_check=True,
                )
                for j in range(nst):
                    sl = slice(j * NHEAD, (j + 1) * NHEAD)
                    nc.tensor.matmul(
                        s_ps[:, sl], lhsT=ht[:, 0, j * 128 : (j + 1) * 128],
                        rhs=v_bf[0], start=False, stop=False,
                        skip_group_check=True,
                    )
                    nc.tensor.matmul(
                        s_ps[:, sl], lhsT=ht[:, 1, j * 128 : (j + 1) * 128],
                        rhs=v_bf[1], start=False, stop=True,
                        skip_group_check=True,
                    )
                # p = exp(leaky_relu(s, 0.2)) in two half-supertile slices so
                # the chain starts after only half the score matmuls. exp is
                # compact [128, nw] on ACT; the class-window broadcast happens
                # inside the DVE multiply via a stride-0 input AP.
                t02 = sg.tile([128, s_tiles * NHEAD], f32, tag="t02")
                slr = sg.tile([128, s_tiles * NHEAD], f32, tag="slr")
                prep = sg.tile([128, s_tiles * NHEAD], bf16, tag="prep")
                ow = sb.tile([128, s_tiles, NHEAD, GCLS], bf16, tag="ow")
                for j0, j1 in ((0, nh1), (nh1, nst)):
                    if j1 <= j0:
                        continue
                    w0, w1 = j0 * NHEAD, j1 * NHEAD
                    nc.vector.tensor_scalar_mul(
                        t02[:, w0:w1], s_ps[:, w0:w1], 0.2
                    )
                    nc.vector.tensor_tensor(
                        out=slr[:, w0:w1], in0=s_ps[:, w0:w1],
                        in1=t02[:, w0:w1], op=OP.max,
                    )
                    nc.scalar.activation(
                        prep[:, w0:w1], slr[:, w0:w1], AF.Exp
                    )
                    nc.vector.tensor_tensor(
                        out=ow[:, j0:j1],
                        in0=mask[:, j0:j1, :]
                        .to_broadcast([128, j1 - j0, GCLS, NHEAD])
                        .rearrange("p j w h -> p j h w"),
                        in1=prep[:, w0:w1]
                        .rearrange("p (j h) -> p j h", h=NHEAD)
                        .to_broadcast([128, j1 - j0, NHEAD, GCLS]),
                        op=OP.mult,
                    )
                ow_tiles[s_] = ow

            def pools(s_):
                ht, hns, nst = stream_tiles.pop(s_)
                ow = ow_tiles.pop(s_)
                ts0, _ = sup_range(s_)
                accs = []
                ended = []
                for j in range(nst):
                    t_ = ts0 + j
                    g, first, last = grp_of[t_]
                    if first:
                        acc_state["t"] = accp.tile(
                            [128, BANKW], f32, tag="gacc", name="gacc"
                        )
                    acc = acc_state["t"]
                    accs.append(acc)
                    rhs = ow[:, j]
                    nc.tensor.matmul(
                        acc[:, 0:128], lhsT=hns[:, j, 0:128], rhs=rhs,
                        start=first, stop=last, skip_group_check=True,
                    )
                    nc.tensor.matmul(
                        acc[:, 128:256], lhsT=hns[:, j, 128:256], rhs=rhs,
                        start=False, stop=last, skip_group_check=True,
                    )
                    if last:
                        ended.append((g, acc))
                # hoisted denom block: ones stays the stationary operand for
                # nst back-to-back matmuls (one weight load)
                for j in range(nst):
                    t_ = ts0 + j
                    g, first, last = grp_of[t_]
                    nc.tensor.matmul(
                        accs[j][0:1, 256:384], lhsT=ones_col[:], rhs=ow[:, j],
                        start=False, stop=last, skip_group_check=True,
                    )
                for g, acc in ended:
                    nc.scalar.activation(
                        out_sb[:, g * 384 : (g + 1) * 384],
                        acc[:, 0:384], AF.Copy,
                    )
                    nc.sync.dma_start(
                        out=out[:, g * 384 : (g + 1) * 384],
                        in_=out_sb[:, g * 384 : (g + 1) * 384],
                    )

            for s_ in range(min(3, nsup)):
                load_and_scores(s_)
            for s_ in range(nsup):
                if s_ + 3 < nsup:
                    load_and_scores(s_ + 3)
                pools(s_)

    nc.compile()
    return nc


def _prep_inputs(context_h, W_lin, b_lin, W_att, b_att, context_y):
    """Host-side shard: argsort by class, 8 class ranges of 125, each split
    into 4 groups of <=32 classes, nodes padded to whole 128-node tiles."""
    h = np.asarray(context_h, dtype=np.float32)
    hb = h.astype(BF)
    y = np.asarray(context_y).astype(np.int64)
    order = np.argsort(y, kind="stable")
    ys = y[order]

    gb = []
    for k in range(NCORES):
        for g in range(NGRP):
            gb.append(k * CPC + min(g * GCLS, CPC))
    gb.append(NCLS)
    gbounds = np.searchsorted(ys, np.asarray(gb))
    cnts = (gbounds[1:] - gbounds[:-1]).reshape(NCORES, NGRP)
    tg = tuple(int(-(-int(cnts[:, g].max()) // 128)) for g in range(NGRP))
    t_tiles = int(sum(tg))

    # fold V[k,h], c[h] from W_lin/W_att/b_lin/b_att (weights-only preproc)
    W_lin = np.asarray(W_lin, dtype=np.float32)
    W_att = np.asarray(W_att, dtype=np.float32)
    V = np.einsum("hok,o->kh", W_lin.reshape(NHEAD, OUTC, INC), W_att)  # [256,4]
    c = (
        np.asarray(b_lin, dtype=np.float32).reshape(NHEAD, OUTC) @ W_att
        + np.asarray(b_att, dtype=np.float32)[0]
    )  # [4]
    cst = np.zeros((128, 44), dtype=BF)
    cst[:, 0:GCLS] = np.arange(GCLS, dtype=np.float32)[None, :].astype(BF)
    cst[:, 32:36] = V[0:128].astype(BF)
    cst[:, 36:40] = V[128:256].astype(BF)
    cst[0, 40:44] = c.astype(BF)

    in_maps = []
    for k in range(NCORES):
        hp = np.zeros((t_tiles * 128, NCOL), dtype=BF)
        hp[:, INC] = BF(float(GCLS + 95))  # pad marker 127 -> mask==0
        row = 0
        for g in range(NGRP):
            gi = k * NGRP + g
            lo, hi = gbounds[gi], gbounds[gi + 1]
            cnt = hi - lo
            idx = order[lo:hi]
            hp[row : row + cnt, 0:INC] = hb[idx]
            hp[row : row + cnt, INC] = (
                ys[lo:hi] - (k * CPC + g * GCLS)
            ).astype(np.float32).astype(BF)
            row += tg[g] * 128
        hn = np.ascontiguousarray(
            hp.reshape(t_tiles, 128, NCOL).transpose(1, 0, 2)
        )
        htr = np.ascontiguousarray(
            hp[:, 0:INC].reshape(t_tiles * 128, 2, 128).transpose(2, 1, 0)
        )
        in_maps.append({"hn": hn, "htr": htr, "cst": cst})
    return in_maps, tg


def kernel(context_h, W_lin, b_lin, W_att, b_att, context_y, num_classes):
    global LAST_RESULT, LAST_PROFILE
    import os

    assert int(num_classes) == NCLS

    from concourse.bass_utils import run_bass_kernel_spmd

    in_maps, tg = _prep_inputs(
        context_h, W_lin, b_lin, W_att, b_att, context_y
    )
    if tg not in _PROG_CACHE:
        _PROG_CACHE[tg] = build_program(tg)
    nc = _PROG_CACHE[tg]
    core_ids = list(range(NCORES))
    res = run_bass_kernel_spmd(nc, in_maps, core_ids)
    LAST_RESULT = res

    if os.environ.get("KERNEL_PROFILE") == "1":
        LAST_PROFILE = run_bass_kernel_spmd(nc, in_maps, core_ids, trace=True)

    outp = np.empty((NCLS, NHEAD * INC), dtype=np.float32)
    for k in range(NCORES):
        o = np.asarray(res.results[k]["outp"])  # [128, 4*384]
        for g in range(NGRP):
            blk = o[:, g * 384 : (g + 1) * 384]
            ncls_g = min(GCLS, CPC - g * GCLS)
            c0 = blk[:, 0:128].reshape(128, NHEAD, GCLS)
            c1 = blk[:, 128:256].reshape(128, NHEAD, GCLS)
            den = blk[0, 256:384].reshape(NHEAD, GCLS)
            den = np.where(den != 0.0, den, 1.0)
            pooled = np.concatenate([c0, c1], axis=0)  # [256, h, w]
            pooled = pooled / den[None, :, :]
            pw = pooled.transpose(2, 1, 0).reshape(GCLS, NHEAD * INC)
            base = k * CPC + g * GCLS
            outp[base : base + ncls_g] = pw[:ncls_g]
    return outp


# revision 23
# speedup vs baseline: 2.2089x; 1.1350x over previous
"""Trainium2 Bass kernel for nn_AttentionPool (gnn_message_passing).

Strategy (v4: class-group windowed pooling, supertile-batched one-hot)
----------------------------------------------------------------------
Math restructure (exactly equivalent to the reference up to fp rounding):
  score[n,h] = context_h[n,:] @ V[:,h] + c[h]        (V, c host-folded from
               W_lin/W_att/b_lin/b_att: V[k,h] = sum_o W_lin[h*128+o,k]W_att[o],
               c[h] = b_lin[h*128:(h+1)*128].W_att + b_att)
  p = exp(leaky_relu(score, 0.2))                     (skip segment-max: scores
               are O(1) so exp cannot overflow; softmax is shift-invariant)
  denom[cls,h]  = sum_{n: y=cls} p[n,h]
  pooled[cls,h,:] = sum_{n: y=cls} p[n,h]*context_h[n,:] / denom[cls,h]

Sharding: BY CLASS. Host argsorts context_y; core k owns classes
[125k, 125k+125) -> no cross-core reduction.

Within a core, classes split into 4 GROUPS of <=32 consecutive classes; the
sorted nodes of each group are padded to whole 128-node tiles, so every tile
touches one 32-class window and the scatter-add matmul runs "flipped":

  per tile j (128 nodes) in group g:
    ow[n, h*32+w] = (yg[n]==w) * p[n,h]
    acc_g[c,    hw] += hn_j[:, 0:128].T @ ow      (feat chunk 0)
    acc_g[c+.., hw] += hn_j[:,128:256].T @ ow     (feat chunk 1)
    acc_g[0(d), hw] += ones.T         @ ow        (denom row)
  acc_g = one PSUM bank (cols 0:128 chunk0, 128:256 chunk1, 256:384 denom);
  ACT flushes it to SBUF at group end, then it's DMA'd out immediately.

ow is built per 16-tile supertile in 2 wide DVE ops (+1 ACT op):
  mask[n,j,w] = (yg[n,j] == iota[w])        one TT is_equal w/ broadcast APs
  prep[n,j,h,w] = Exp(slr[n,j,h])           ACT exp, stride-0 bcast over w
  ow = mask (bcast over h) * prep           one TT mult
which sidesteps the ~190ns fixed cost of narrow per-tile DVE ops.

DMA: host pre-lays-out everything contiguous; hn carries yg as column 256;
h^T goes on the ACT HWDGE ring, hn on the SP ring; constants are one DMA;
each group's result is DMA'd out as soon as it's flushed.
"""

import sys

sys.path.insert(0, "/opt/trn_rl_repo")

import numpy as np
import ml_dtypes

BF = ml_dtypes.bfloat16

N = 100000
INC = 256
NHEAD = 4
OUTC = 128
NCLS = 1000
NCORES = 8
CPC = NCLS // NCORES  # 125 classes per core
GCLS = 32  # classes per group window
NGRP = 4  # groups per core (32+32+32+29)
NCOL = INC + 2  # hn row: 256 features, yg, zero pad
BANKW = 512  # one PSUM bank; 0:128 c0, 128:256 c1, 256:384 denom

_PROG_CACHE = {}
LAST_RESULT = None
LAST_PROFILE = None


def build_program(tg, s_tiles=16):
    """Build + compile the SPMD Bass program. tg = tiles per group (len 4)."""
    from concourse import bacc, mybir, tile

    f32 = mybir.dt.float32
    bf16 = mybir.dt.bfloat16
    AF = mybir.ActivationFunctionType
    OP = mybir.AluOpType

    t_tiles = int(sum(tg))
    cap = t_tiles * 128
    grp_of = []
    for g, tcnt in enumerate(tg):
        for i in range(tcnt):
            grp_of.append((g, i == 0, i == tcnt - 1))

    nc = bacc.Bacc(
        "TRN2", target_bir_lowering=False, debug=False, num_devices=NCORES
    )

    hn = nc.dram_tensor("hn", [128, t_tiles, NCOL], bf16, kind="ExternalInput").ap()
    htr = nc.dram_tensor("htr", [128, 2, cap], bf16, kind="ExternalInput").ap()
    # consts: cols 0:32 iota, 32:40 [V|0.2V] chunk0, 40:48 chunk1,
    # row0 48:56 [c|0.2c]
    cst = nc.dram_tensor("cst", [128, 64], bf16, kind="ExternalInput").ap()
    out = nc.dram_tensor(
        "outp", [128, NGRP * 384], f32, kind="ExternalOutput"
    ).ap()

    # supertile schedule: small ones first so the pipeline primes while the
    # first DMAs stream, then full 16-tile supertiles
    sizes = []
    rem = t_tiles
    for sz in (2, 4, 8):
        if rem > sz + s_tiles:
            sizes.append(sz)
            rem -= sz
    while rem > 0:
        sizes.append(min(s_tiles, rem))
        rem -= sizes[-1]
    starts = []
    acc0 = 0
    for sz in sizes:
        starts.append(acc0)
        acc0 += sz
    nsup = len(sizes)

    def sup_range(s_):
        return starts[s_], sizes[s_]

    with tile.TileContext(nc) as tc:
        with (
            tc.tile_pool(name="const", bufs=1) as cpool,
            tc.tile_pool(name="stream", bufs=4) as sb,
            tc.tile_pool(name="work", bufs=3) as sg,
            tc.tile_pool(name="ps", bufs=4, space="PSUM") as ps,
            tc.tile_pool(name="acc", bufs=2, space="PSUM") as accp,
        ):
            # ---- constants (one DMA) --------------------------------------
            cst_sb = cpool.tile([128, 64], bf16)
            nc.sync.dma_start(out=cst_sb[:], in_=cst)
            ciota_sb = cst_sb[:, 0:GCLS]
            v_bf = [cst_sb[:, 32:40], cst_sb[:, 40:48]]
            ones_row = cpool.tile([1, 128], bf16)
            nc.vector.memset(ones_row[:1], 1.0)
            ones_col = cpool.tile([128, 1], bf16)
            nc.vector.memset(ones_col[:], 1.0)
            c_rep = cpool.tile([1, s_tiles * 2 * NHEAD], bf16)
            nc.vector.tensor_copy(
                out=c_rep[:1].rearrange("p (j h) -> p j h", h=2 * NHEAD),
                in_=cst_sb[0:1, 48:56]
                .to_broadcast([1, 2 * NHEAD, s_tiles])
                .rearrange("p h j -> p j h"),
            )

            out_sb = cpool.tile([128, NGRP * 384], f32)

            stream_tiles = {}
            ow_tiles = {}
            acc_state = {"t": None}

            def load_and_scores(s_):
                ts0, nst = sup_range(s_)
                c0 = ts0 * 128
                c1 = c0 + nst * 128
                nh1 = (nst + 1) // 2  # first-half tiles
                # h^T chunks on the ACT HWDGE ring; node-major on the SP ring
                ht = sb.tile([128, 2, s_tiles * 128], bf16, tag="ht")
                nc.scalar.dma_start(
                    out=ht[:, :, : nst * 128], in_=htr[:, :, c0:c1]
                )
                hns = sb.tile([128, s_tiles, NCOL], bf16, tag="hn")
                nc.sync.dma_start(out=hns[:, :nst, :], in_=hn[:, ts0 : ts0 + nst, :])
                stream_tiles[s_] = (ht, hns, nst)

                # mask is off the score critical path (needs only the DMA)
                mask = sg.tile([128, s_tiles, GCLS], bf16, tag="mask")
                nc.vector.tensor_tensor(
                    out=mask[:, :nst, :],
                    in0=hns[:, :nst, INC].to_broadcast([128, nst, GCLS]),
                    in1=ciota_sb.to_broadcast([128, GCLS, nst]).rearrange(
                        "p w j -> p j w"
                    ),
                    op=OP.is_equal,
                )

                # scores AND 0.2*scores in one pass (rhs = [V | 0.2V]) into
                # one full (padded) PSUM bank, so leaky is a single DVE max
                s_ps = ps.tile(
                    [128, 512], f32, tag="sps", padded_shape=[128, 512]
                )
                nw8 = nst * 2 * NHEAD
                nc.tensor.matmul(
                    s_ps[:, :nw8], lhsT=ones_row[:1], rhs=c_rep[:1, :nw8],
                    start=True, stop=False, skip_group_check=True,
                )
                for j in range(nst):
                    sl = slice(j * 2 * NHEAD, (j + 1) * 2 * NHEAD)
                    nc.tensor.matmul(
                        s_ps[:, sl], lhsT=ht[:, 0, j * 128 : (j + 1) * 128],
                        rhs=v_bf[0], start=False, stop=False,
                        skip_group_check=True,
                    )
                    nc.tensor.matmul(
                        s_ps[:, sl], lhsT=ht[:, 1, j * 128 : (j + 1) * 128],
                        rhs=v_bf[1], start=False, stop=True,
                        skip_group_check=True,
                    )
                # p = exp(max(s, 0.2s)) in two half-supertile slices so the
                # chain starts after only half the score matmuls. exp is
                # compact [128, nw] on ACT; the class-window broadcast happens
                # inside the DVE multiply via a stride-0 input AP.
                spv = s_ps[:, : nst * 2 * NHEAD].rearrange(
                    "p (j h t) -> p j h t", h=NHEAD, t=2
                )
                slr = sg.tile([128, s_tiles * NHEAD], f32, tag="slr")
                prep = sg.tile([128, s_tiles * NHEAD], bf16, tag="prep")
                ow = sb.tile([128, s_tiles, NHEAD, GCLS], bf16, tag="ow")
                for j0, j1 in ((0, nh1), (nh1, nst)):
                    if j1 <= j0:
                        continue
                    w0, w1 = j0 * NHEAD, j1 * NHEAD
                    nc.vector.tensor_reduce(
                        out=slr[:, w0:w1].rearrange(
                            "p (j h) -> p j h", h=NHEAD
                        ),
                        in_=spv[:, j0:j1],
                        axis=mybir.AxisListType.X,
                        op=OP.max,
                    )
                    nc.scalar.activation(
                        prep[:, w0:w1], slr[:, w0:w1], AF.Exp
                    )
                    nc.vector.tensor_tensor(
                        out=ow[:, j0:j1],
                        in0=mask[:, j0:j1, :]
                        .to_broadcast([128, j1 - j0, GCLS, NHEAD])
                        .rearrange("p j w h -> p j h w"),
                        in1=prep[:, w0:w1]
                        .rearrange("p (j h) -> p j h", h=NHEAD)
                        .to_broadcast([128, j1 - j0, NHEAD, GCLS]),
                        op=OP.mult,
                    )
                ow_tiles[s_] = ow

            def pools(s_):
                ht, hns, nst = stream_tiles.pop(s_)
                ow = ow_tiles.pop(s_)
                ts0, _ = sup_range(s_)
                accs = []
                ended = []
                for j in range(nst):
                    t_ = ts0 + j
                    g, first, last = grp_of[t_]
                    if first:
                        acc_state["t"] = accp.tile(
                            [128, BANKW], f32, tag="gacc", name="gacc"
                        )
                    acc = acc_state["t"]
                    accs.append(acc)
                    rhs = ow[:, j]
                    nc.tensor.matmul(
                        acc[:, 0:128], lhsT=hns[:, j, 0:128], rhs=rhs,
                        start=first, stop=last, skip_group_check=True,
                    )
                    nc.tensor.matmul(
                        acc[:, 128:256], lhsT=hns[:, j, 128:256], rhs=rhs,
                        start=False, stop=last, skip_group_check=True,
                    )
                    if last:
                        ended.append((g, acc))
                # hoisted denom block: ones stays the stationary operand for
                # nst back-to-back matmuls (one weight load)
                for j in range(nst):
                    t_ = ts0 + j
                    g, first, last = grp_of[t_]
                    nc.tensor.matmul(
                        accs[j][0:1, 256:384], lhsT=ones_col[:], rhs=ow[:, j],
                        start=False, stop=last, skip_group_check=True,
                    )
                for g, acc in ended:
                    nc.scalar.activation(
                        out_sb[:, g * 384 : (g + 1) * 384],
                        acc[:, 0:384], AF.Copy,
                    )
                    nc.sync.dma_start(
                        out=out[:, g * 384 : (g + 1) * 384],
                        in_=out_sb[:, g * 384 : (g + 1) * 384],
                    )

            for s_ in range(min(3, nsup)):
                load_and_scores(s_)
            for s_ in range(nsup):
                if s_ + 3 < nsup:
                    load_and_scores(s_ + 3)
                pools(s_)

    nc.compile()
    return nc


def _prep_inputs(context_h, W_lin, b_lin, W_att, b_att, context_y):
    """Host-side shard: argsort by class, 8 class ranges of 125, each split
    into 4 groups of <=32 classes, nodes padded to whole 128-node tiles."""
    h = np.asarray(context_h, dtype=np.float32)
    hb = h.astype(BF)
    y = np.asarray(context_y).astype(np.int64)
    order = np.argsort(y, kind="stable")
    ys = y[order]

    gb = []
    for k in range(NCORES):
        for g in range(NGRP):
            gb.append(k * CPC + min(g * GCLS, CPC))
    gb.append(NCLS)
    gbounds = np.searchsorted(ys, np.asarray(gb))
    cnts = (gbounds[1:] - gbounds[:-1]).reshape(NCORES, NGRP)
    tg = tuple(int(-(-int(cnts[:, g].max()) // 128)) for g in range(NGRP))
    t_tiles = int(sum(tg))

    # fold V[k,h], c[h] from W_lin/W_att/b_lin/b_att (weights-only preproc)
    W_lin = np.asarray(W_lin, dtype=np.float32)
    W_att = np.asarray(W_att, dtype=np.float32)
    V = np.einsum("hok,o->kh", W_lin.reshape(NHEAD, OUTC, INC), W_att)  # [256,4]
    c = (
        np.asarray(b_lin, dtype=np.float32).reshape(NHEAD, OUTC) @ W_att
        + np.asarray(b_att, dtype=np.float32)[0]
    )  # [4]
    cst = np.zeros((128, 64), dtype=BF)
    cst[:, 0:GCLS] = np.arange(GCLS, dtype=np.float32)[None, :].astype(BF)
    # [V | 0.2V] interleaved (h, t): col 2h = V[:,h], col 2h+1 = 0.2V[:,h]
    v8 = np.stack([V, 0.2 * V], axis=2).reshape(INC, 2 * NHEAD)
    cst[:, 32:40] = v8[0:128].astype(BF)
    cst[:, 40:48] = v8[128:256].astype(BF)
    c8 = np.stack([c, 0.2 * c], axis=1).reshape(2 * NHEAD)
    cst[0, 48:56] = c8.astype(BF)

    in_maps = []
    for k in range(NCORES):
        hp = np.zeros((t_tiles * 128, NCOL), dtype=BF)
        hp[:, INC] = BF(float(GCLS + 95))  # pad marker 127 -> mask==0
        row = 0
        for g in range(NGRP):
            gi = k * NGRP + g
            lo, hi = gbounds[gi], gbounds[gi + 1]
            cnt = hi - lo
            idx = order[lo:hi]
            hp[row : row + cnt, 0:INC] = hb[idx]
            hp[row : row + cnt, INC] = (
                ys[lo:hi] - (k * CPC + g * GCLS)
            ).astype(np.float32).astype(BF)
            row += tg[g] * 128
        hn = np.ascontiguousarray(
            hp.reshape(t_tiles, 128, NCOL).transpose(1, 0, 2)
        )
        htr = np.ascontiguousarray(
            hp[:, 0:INC].reshape(t_tiles * 128, 2, 128).transpose(2, 1, 0)
        )
        in_maps.append({"hn": hn, "htr": htr, "cst": cst})
    return in_maps, tg


def kernel(context_h, W_lin, b_lin, W_att, b_att, context_y, num_classes):
    global LAST_RESULT, LAST_PROFILE
    import os

    assert int(num_classes) == NCLS

    from concourse.bass_utils import run_bass_kernel_spmd

    in_maps, tg = _prep_inputs(
        context_h, W_lin, b_lin, W_att, b_att, context_y
    )
    if tg not in _PROG_CACHE:
        _PROG_CACHE[tg] = build_program(tg)
    nc = _PROG_CACHE[tg]
    core_ids = list(range(NCORES))
    res = run_bass_kernel_spmd(nc, in_maps, core_ids)
    LAST_RESULT = res

    if os.environ.get("KERNEL_PROFILE") == "1":
        LAST_PROFILE = run_bass_kernel_spmd(nc, in_maps, core_ids, trace=True)

    outp = np.empty((NCLS, NHEAD * INC), dtype=np.float32)
    for k in range(NCORES):
        o = np.asarray(res.results[k]["outp"])  # [128, 4*384]
        for g in range(NGRP):
            blk = o[:, g * 384 : (g + 1) * 384]
            ncls_g = min(GCLS, CPC - g * GCLS)
            c0 = blk[:, 0:128].reshape(128, NHEAD, GCLS)
            c1 = blk[:, 128:256].reshape(128, NHEAD, GCLS)
            den = blk[0, 256:384].reshape(NHEAD, GCLS)
            den = np.where(den != 0.0, den, 1.0)
            pooled = np.concatenate([c0, c1], axis=0)  # [256, h, w]
            pooled = pooled / den[None, :, :]
            pw = pooled.transpose(2, 1, 0).reshape(GCLS, NHEAD * INC)
            base = k * CPC + g * GCLS
            outp[base : base + ncls_g] = pw[:ncls_g]
    return outp


# revision 25
# speedup vs baseline: 2.4753x; 1.1206x over previous
"""Trainium2 Bass kernel for nn_AttentionPool (gnn_message_passing).

Strategy (v4: class-group windowed pooling, supertile-batched one-hot)
----------------------------------------------------------------------
Math restructure (exactly equivalent to the reference up to fp rounding):
  score[n,h] = context_h[n,:] @ V[:,h] + c[h]        (V, c host-folded from
               W_lin/W_att/b_lin/b_att: V[k,h] = sum_o W_lin[h*128+o,k]W_att[o],
               c[h] = b_lin[h*128:(h+1)*128].W_att + b_att)
  p = exp(leaky_relu(score, 0.2))                     (skip segment-max: scores
               are O(1) so exp cannot overflow; softmax is shift-invariant)
  denom[cls,h]  = sum_{n: y=cls} p[n,h]
  pooled[cls,h,:] = sum_{n: y=cls} p[n,h]*context_h[n,:] / denom[cls,h]

Sharding: BY CLASS. Host argsorts context_y; core k owns classes
[125k, 125k+125) -> no cross-core reduction.

Within a core, classes split into 4 GROUPS of <=32 consecutive classes; the
sorted nodes of each group are padded to whole 128-node tiles, so every tile
touches one 32-class window and the scatter-add matmul runs "flipped":

  per tile j (128 nodes) in group g:
    ow[n, h*32+w] = (yg[n]==w) * p[n,h]
    acc_g[c,    hw] += hn_j[:, 0:128].T @ ow      (feat chunk 0)
    acc_g[c+.., hw] += hn_j[:,128:256].T @ ow     (feat chunk 1)
    acc_g[0(d), hw] += ones.T         @ ow        (denom row)
  acc_g = one PSUM bank (cols 0:128 chunk0, 128:256 chunk1, 256:384 denom);
  ACT flushes it to SBUF at group end, then it's DMA'd out immediately.

ow is built per 16-tile supertile in 2 wide DVE ops (+1 ACT op):
  mask[n,j,w] = (yg[n,j] == iota[w])        one TT is_equal w/ broadcast APs
  prep[n,j,h,w] = Exp(slr[n,j,h])           ACT exp, stride-0 bcast over w
  ow = mask (bcast over h) * prep           one TT mult
which sidesteps the ~190ns fixed cost of narrow per-tile DVE ops.

DMA: host pre-lays-out everything contiguous; hn carries yg as column 256;
h^T goes on the ACT HWDGE ring, hn on the SP ring; constants are one DMA;
each group's result is DMA'd out as soon as it's flushed.
"""

import sys

sys.path.insert(0, "/opt/trn_rl_repo")

import numpy as np
import ml_dtypes

BF = ml_dtypes.bfloat16

N = 100000
INC = 256
NHEAD = 4
OUTC = 128
NCLS = 1000
NCORES = 8
CPC = NCLS // NCORES  # 125 classes per core
GCLS = 32  # classes per group window
NGRP = 4  # groups per core (32+32+32+29)
NCOL = INC + 2  # hn row: 256 features, yg, zero pad
BANKW = 512  # one PSUM bank; 0:128 c0, 128:256 c1, 256:384 denom

_PROG_CACHE = {}
LAST_RESULT = None
LAST_PROFILE = None


def build_program(tg, s_tiles=16):
    """Build + compile the SPMD Bass program. tg = tiles per group (len 4)."""
    from concourse import bacc, mybir, tile

    f32 = mybir.dt.float32
    bf16 = mybir.dt.bfloat16
    AF = mybir.ActivationFunctionType
    OP = mybir.AluOpType

    t_tiles = int(sum(tg))
    cap = t_tiles * 128
    grp_of = []
    for g, tcnt in enumerate(tg):
        for i in range(tcnt):
            grp_of.append((g, i == 0, i == tcnt - 1))

    nc = bacc.Bacc(
        "TRN2", target_bir_lowering=False, debug=False, num_devices=NCORES
    )

    hn = nc.dram_tensor("hn", [128, t_tiles, NCOL], bf16, kind="ExternalInput").ap()
    htr = nc.dram_tensor("htr", [128, 2, cap], bf16, kind="ExternalInput").ap()
    # consts: cols 0:32 iota, 32:40 [V|0.2V] chunk0, 40:48 chunk1,
    # row0 48:56 [c|0.2c]
    cst = nc.dram_tensor("cst", [128, 64], bf16, kind="ExternalInput").ap()
    out = nc.dram_tensor(
        "outp", [128, NGRP * 384], f32, kind="ExternalOutput"
    ).ap()

    # supertile schedule: small ones first so the pipeline primes while the
    # first DMAs stream, then full 16-tile supertiles
    sizes = []
    rem = t_tiles
    for sz in (2, 4, 8):
        if rem > sz + s_tiles:
            sizes.append(sz)
            rem -= sz
    while rem > 0:
        sizes.append(min(s_tiles, rem))
        rem -= sizes[-1]
    starts = []
    acc0 = 0
    for sz in sizes:
        starts.append(acc0)
        acc0 += sz
    nsup = len(sizes)

    def sup_range(s_):
        return starts[s_], sizes[s_]

    with tile.TileContext(nc) as tc:
        with (
            tc.tile_pool(name="const", bufs=1) as cpool,
            tc.tile_pool(name="stream", bufs=5) as sb,
            tc.tile_pool(name="work", bufs=4) as sg,
            tc.tile_pool(name="ps", bufs=5, space="PSUM") as ps,
            tc.tile_pool(name="acc", bufs=2, space="PSUM") as accp,
        ):
            # ---- constants (one DMA) --------------------------------------
            cst_sb = cpool.tile([128, 64], bf16)
            nc.sync.dma_start(out=cst_sb[:], in_=cst)
            ciota_sb = cst_sb[:, 0:GCLS]
            v_bf = [cst_sb[:, 32:40], cst_sb[:, 40:48]]
            ones_row = cpool.tile([1, 128], bf16)
            nc.vector.memset(ones_row[:1], 1.0)
            ones_col = cpool.tile([128, 1], bf16)
            nc.vector.memset(ones_col[:], 1.0)
            c_rep = cpool.tile([1, s_tiles * 2 * NHEAD], bf16)
            nc.vector.tensor_copy(
                out=c_rep[:1].rearrange("p (j h) -> p j h", h=2 * NHEAD),
                in_=cst_sb[0:1, 48:56]
                .to_broadcast([1, 2 * NHEAD, s_tiles])
                .rearrange("p h j -> p j h"),
            )

            out_sb = cpool.tile([128, NGRP * 384], f32)

            stream_tiles = {}
            ow_tiles = {}
            acc_state = {"t": None}

            def load_and_scores(s_):
                ts0, nst = sup_range(s_)
                c0 = ts0 * 128
                c1 = c0 + nst * 128
                nh1 = (nst + 1) // 2  # first-half tiles
                # h^T chunks on the ACT HWDGE ring; node-major on the SP ring
                ht = sb.tile([128, 2, s_tiles * 128], bf16, tag="ht")
                nc.scalar.dma_start(
                    out=ht[:, :, : nst * 128], in_=htr[:, :, c0:c1]
                )
                hns = sb.tile([128, s_tiles, NCOL], bf16, tag="hn")
                nc.sync.dma_start(out=hns[:, :nst, :], in_=hn[:, ts0 : ts0 + nst, :])
                stream_tiles[s_] = (ht, hns, nst)

                # mask is off the score critical path (needs only the DMA)
                mask = sg.tile([128, s_tiles, GCLS], bf16, tag="mask")
                nc.vector.tensor_tensor(
                    out=mask[:, :nst, :],
                    in0=hns[:, :nst, INC].to_broadcast([128, nst, GCLS]),
                    in1=ciota_sb.to_broadcast([128, GCLS, nst]).rearrange(
                        "p w j -> p j w"
                    ),
                    op=OP.is_equal,
                )

                # scores AND 0.2*scores in one pass (rhs = [V | 0.2V]) into
                # one full (padded) PSUM bank, so leaky is a single DVE max
                s_ps = ps.tile(
                    [128, 512], f32, tag="sps", padded_shape=[128, 512]
                )
                nw8 = nst * 2 * NHEAD
                nc.tensor.matmul(
                    s_ps[:, :nw8], lhsT=ones_row[:1], rhs=c_rep[:1, :nw8],
                    start=True, stop=False, skip_group_check=True,
                )
                for j in range(nst):
                    sl = slice(j * 2 * NHEAD, (j + 1) * 2 * NHEAD)
                    nc.tensor.matmul(
                        s_ps[:, sl], lhsT=ht[:, 0, j * 128 : (j + 1) * 128],
                        rhs=v_bf[0], start=False, stop=False,
                        skip_group_check=True,
                    )
                    nc.tensor.matmul(
                        s_ps[:, sl], lhsT=ht[:, 1, j * 128 : (j + 1) * 128],
                        rhs=v_bf[1], start=False, stop=True,
                        skip_group_check=True,
                    )
                # p = exp(max(s, 0.2s)) in two half-supertile slices so the
                # chain starts after only half the score matmuls. exp is
                # compact [128, nw] on ACT; the class-window broadcast happens
                # inside the DVE multiply via a stride-0 input AP.
                spv = s_ps[:, : nst * 2 * NHEAD].rearrange(
                    "p (j h t) -> p j h t", h=NHEAD, t=2
                )
                slr = sg.tile([128, s_tiles * NHEAD], f32, tag="slr")
                prep = sg.tile([128, s_tiles * NHEAD], bf16, tag="prep")
                ow = sb.tile([128, s_tiles, NHEAD, GCLS], bf16, tag="ow")
                for j0, j1 in ((0, nh1), (nh1, nst)):
                    if j1 <= j0:
                        continue
                    w0, w1 = j0 * NHEAD, j1 * NHEAD
                    nc.vector.tensor_reduce(
                        out=slr[:, w0:w1].rearrange(
                            "p (j h) -> p j h", h=NHEAD
                        ),
                        in_=spv[:, j0:j1],
                        axis=mybir.AxisListType.X,
                        op=OP.max,
                    )
                    nc.scalar.activation(
                        prep[:, w0:w1], slr[:, w0:w1], AF.Exp
                    )
                    nc.vector.tensor_tensor(
                        out=ow[:, j0:j1],
                        in0=mask[:, j0:j1, :]
                        .to_broadcast([128, j1 - j0, GCLS, NHEAD])
                        .rearrange("p j w h -> p j h w"),
                        in1=prep[:, w0:w1]
                        .rearrange("p (j h) -> p j h", h=NHEAD)
                        .to_broadcast([128, j1 - j0, NHEAD, GCLS]),
                        op=OP.mult,
                    )
                ow_tiles[s_] = ow

            def pools(s_):
                ht, hns, nst = stream_tiles.pop(s_)
                ow = ow_tiles.pop(s_)
                ts0, _ = sup_range(s_)
                accs = []
                ended = []
                for j in range(nst):
                    t_ = ts0 + j
                    g, first, last = grp_of[t_]
                    if first:
                        acc_state["t"] = accp.tile(
                            [128, BANKW], f32, tag="gacc", name="gacc"
                        )
                    acc = acc_state["t"]
                    accs.append(acc)
                    rhs = ow[:, j]
                    nc.tensor.matmul(
                        acc[:, 0:128], lhsT=hns[:, j, 0:128], rhs=rhs,
                        start=first, stop=last, skip_group_check=True,
                    )
                    nc.tensor.matmul(
                        acc[:, 128:256], lhsT=hns[:, j, 128:256], rhs=rhs,
                        start=False, stop=last, skip_group_check=True,
                    )
                    if last:
                        ended.append((g, acc))
                # hoisted denom block: ones stays the stationary operand for
                # nst back-to-back matmuls (one weight load)
                for j in range(nst):
                    t_ = ts0 + j
                    g, first, last = grp_of[t_]
                    nc.tensor.matmul(
                        accs[j][0:1, 256:384], lhsT=ones_col[:], rhs=ow[:, j],
                        start=False, stop=last, skip_group_check=True,
                    )
                for g, acc in ended:
                    nc.scalar.activation(
                        out_sb[:, g * 384 : (g + 1) * 384],
                        acc[:, 0:384], AF.Copy,
                    )
                    # scalar ring: right behind its own flush, and it never
                    # blocks the sync ring's hn loads (head-of-line)
                    nc.scalar.dma_start(
                        out=out[:, g * 384 : (g + 1) * 384],
                        in_=out_sb[:, g * 384 : (g + 1) * 384],
                    )

            for s_ in range(min(4, nsup)):
                load_and_scores(s_)
            for s_ in range(nsup):
                if s_ + 4 < nsup:
                    load_and_scores(s_ + 4)
                pools(s_)

    nc.compile()
    return nc


def _prep_inputs(context_h, W_lin, b_lin, W_att, b_att, context_y):
    """Host-side shard: argsort by class, 8 class ranges of 125, each split
    into 4 groups of <=32 classes, nodes padded to whole 128-node tiles."""
    h = np.asarray(context_h, dtype=np.float32)
    hb = h.astype(BF)
    y = np.asarray(context_y).astype(np.int64)
    order = np.argsort(y, kind="stable")
    ys = y[order]

    gb = []
    for k in range(NCORES):
        for g in range(NGRP):
            gb.append(k * CPC + min(g * GCLS, CPC))
    gb.append(NCLS)
    gbounds = np.searchsorted(ys, np.asarray(gb))
    cnts = (gbounds[1:] - gbounds[:-1]).reshape(NCORES, NGRP)
    tg = tuple(int(-(-int(cnts[:, g].max()) // 128)) for g in range(NGRP))
    t_tiles = int(sum(tg))

    # fold V[k,h], c[h] from W_lin/W_att/b_lin/b_att (weights-only preproc)
    W_lin = np.asarray(W_lin, dtype=np.float32)
    W_att = np.asarray(W_att, dtype=np.float32)
    V = np.einsum("hok,o->kh", W_lin.reshape(NHEAD, OUTC, INC), W_att)  # [256,4]
    c = (
        np.asarray(b_lin, dtype=np.float32).reshape(NHEAD, OUTC) @ W_att
        + np.asarray(b_att, dtype=np.float32)[0]
    )  # [4]
    cst = np.zeros((128, 64), dtype=BF)
    cst[:, 0:GCLS] = np.arange(GCLS, dtype=np.float32)[None, :].astype(BF)
    # [V | 0.2V] interleaved (h, t): col 2h = V[:,h], col 2h+1 = 0.2V[:,h]
    v8 = np.stack([V, 0.2 * V], axis=2).reshape(INC, 2 * NHEAD)
    cst[:, 32:40] = v8[0:128].astype(BF)
    cst[:, 40:48] = v8[128:256].astype(BF)
    c8 = np.stack([c, 0.2 * c], axis=1).reshape(2 * NHEAD)
    cst[0, 48:56] = c8.astype(BF)

    in_maps = []
    for k in range(NCORES):
        hp = np.zeros((t_tiles * 128, NCOL), dtype=BF)
        hp[:, INC] = BF(float(GCLS + 95))  # pad marker 127 -> mask==0
        row = 0
        for g in range(NGRP):
            gi = k * NGRP + g
            lo, hi = gbounds[gi], gbounds[gi + 1]
            cnt = hi - lo
            idx = order[lo:hi]
            hp[row : row + cnt, 0:INC] = hb[idx]
            hp[row : row + cnt, INC] = (
                ys[lo:hi] - (k * CPC + g * GCLS)
            ).astype(np.float32).astype(BF)
            row += tg[g] * 128
        hn = np.ascontiguousarray(
            hp.reshape(t_tiles, 128, NCOL).transpose(1, 0, 2)
        )
        htr = np.ascontiguousarray(
            hp[:, 0:INC].reshape(t_tiles * 128, 2, 128).transpose(2, 1, 0)
        )
        in_maps.append({"hn": hn, "htr": htr, "cst": cst})
    return in_maps, tg


def kernel(context_h, W_lin, b_lin, W_att, b_att, context_y, num_classes):
    global LAST_RESULT, LAST_PROFILE
    import os

    assert int(num_classes) == NCLS

    from concourse.bass_utils import run_bass_kernel_spmd

    in_maps, tg = _prep_inputs(
        context_h, W_lin, b_lin, W_att, b_att, context_y
    )
    if tg not in _PROG_CACHE:
        _PROG_CACHE[tg] = build_program(tg)
    nc = _PROG_CACHE[tg]
    core_ids = list(range(NCORES))
    res = run_bass_kernel_spmd(nc, in_maps, core_ids)
    LAST_RESULT = res

    if os.environ.get("KERNEL_PROFILE") == "1":
        LAST_PROFILE = run_bass_kernel_spmd(nc, in_maps, core_ids, trace=True)

    outp = np.empty((NCLS, NHEAD * INC), dtype=np.float32)
    for k in range(NCORES):
        o = np.asarray(res.results[k]["outp"])  # [128, 4*384]
        for g in range(NGRP):
            blk = o[:, g * 384 : (g + 1) * 384]
            ncls_g = min(GCLS, CPC - g * GCLS)
            c0 = blk[:, 0:128].reshape(128, NHEAD, GCLS)
            c1 = blk[:, 128:256].reshape(128, NHEAD, GCLS)
            den = blk[0, 256:384].reshape(NHEAD, GCLS)
            den = np.where(den != 0.0, den, 1.0)
            pooled = np.concatenate([c0, c1], axis=0)  # [256, h, w]
            pooled = pooled / den[None, :, :]
            pw = pooled.transpose(2, 1, 0).reshape(GCLS, NHEAD * INC)
            base = k * CPC + g * GCLS
            outp[base : base + ncls_g] = pw[:ncls_g]
    return outp
